# revision 7
# baseline (speedup 1.0000x reference)
"""CaNet (moe_routing GNN) forward on 8 Trainium2 NeuronCores.

Sharding: nodes are range-partitioned across the 8 cores (6250 each, padded
to 6272 = 49*128). Each core owns the edges whose *destination* lands in its
node range. The GCN aggregation out[col] += val * h[row] is computed as a
sequence of tiny one-hot matmuls on the TensorEngine:

  - edges are sorted by destination tile (groups of 128 dest nodes) on the
    host and padded to multiples of 128 ("chunks");
  - the source features h[row] for one chunk are fetched from a replicated
    node-major bf16 table in HBM with the GpSimd dma_gather custom op
    (int16 indices; the table is split into a 24576-row "lo" region, the
    first 24 tiles of every core, and a 25600-row "hi" region, the last 25
    tiles of every core -- every chunk draws from one region);
  - the [128e x 128d] selection matrices S (S[e,d] = (d == ldest[e]) * val[e])
    are precomputed on the host (the edge list is a compile-time constant)
    and streamed from HBM with plain HWDGE DMA, one slab per gather call --
    this keeps the DVE out of the aggregation entirely;
  - psum_gcnT[f,d] += G_chunk.T @ S accumulates over the tile's chunks.

The lo/hi table split doubles as a latency hider: the AllGather between
layers is split into two collectives (tiles 0-23 -> tab_lo, tiles 24-48 ->
tab_hi), so the lo-region collective completes while the previous layer's
gathers still run and the next layer's lo gathers start immediately.

Dense per-node work (expert gate softmax, the K=4 expert convs, mixing,
residual relu, fc0/fc1) runs in bf16 matmuls + f32 psum per 128-node tile.

The per-core *program* is identical (SPMD); all per-core variation (gather
indices, S slabs, x slab) arrives via ExternalInputs. Chunk counts per
(tile, half) are max'd across cores so the schedule is static; padding
slots use idx=0 with an all-zero S row and contribute nothing.
"""

import sys

sys.path.insert(0, "/opt/trn_rl_repo")

import numpy as np
import ml_dtypes

import concourse.bacc as bacc
import concourse.tile as tile
import concourse.mybir as mybir
import concourse.bass as bass
from concourse import bass_utils
from concourse.masks import make_identity

# Problem constants (hardcoded per contract).
N = 50000
E = 800000
D = 128  # input dim
H = 128  # hidden dim
C = 47  # classes
K = 4  # experts
L = 2  # conv layers
M = 8  # cores

NPC = N // M  # 6250 nodes per core
T = (NPC + 127) // 128  # 49 tiles per core
NPAD = T * 128  # 6272
TA = 24  # tiles in the "lo" table region per core
TB = T - TA  # 25 tiles in the "hi" region
RA = TA * 128  # 3072 lo rows per core
RB = TB * 128  # 3200 hi rows per core
LOROW = M * RA  # 24576 lo region rows
HIROW = M * RB  # 25600 hi region rows
CH = 8192  # gather indices per dma_gather call
BLK = CH // 128  # 64 chunk blocks per gather call

F32 = mybir.dt.float32
BF16 = mybir.dt.bfloat16
I16 = mybir.dt.int16
BF = ml_dtypes.bfloat16


def _preprocess(x, edge_index, fc0_w, fc0_b, fc1_w, fc1_b, env_w, env_b, conv_w):
    """Host-side: degree/value computation, edge sort, static chunk schedule,
    per-core gather index + S-matrix arrays, weight packing."""
    row = np.asarray(edge_index[0], np.int64)
    col = np.asarray(edge_index[1], np.int64)

    deg = np.bincount(col, minlength=N).astype(np.float32)
    dinv = np.where(deg > 0, 1.0 / np.sqrt(deg), 0.0).astype(np.float32)
    val = (dinv[col] * dinv[row]).astype(np.float32)

    core = col // NPC
    dloc = col % NPC
    tl = dloc // 128
    ld = dloc % 128
    # table row: lo region holds tiles 0..TA-1 of every core, hi the rest
    score = row // NPC
    sloc = row % NPC
    half = (sloc >= RA).astype(np.int64)
    srow = np.where(half == 0, score * RA + sloc, score * RB + (sloc - RA))
    idx16 = srow  # already region-local

    cnt = np.bincount((core * T + tl) * 2 + half, minlength=M * T * 2).reshape(
        M, T, 2
    )
    nch = -(-cnt // 128)  # ceil div, [M, T, 2]
    NCH = nch.max(axis=0)  # [T, 2] static schedule
    assert NCH.sum() > 0
    tot = NCH.sum(axis=0)  # [2] total chunks per stream
    ncall = [int(-(-int(tot[s]) * 128 // CH)) for s in range(2)]
    totpad = [ncall[s] * BLK for s in range(2)]  # chunks incl. call padding

    base = np.zeros((T, 2), np.int64)
    base[1:] = NCH[:-1].cumsum(axis=0)

    gkey = (core * 2 + half) * T + tl
    order = np.argsort(gkey, kind="stable")
    gsort = gkey[order]
    starts = np.searchsorted(gsort, np.arange(M * 2 * T))
    rank = np.arange(E, dtype=np.int64) - starts[gsort]
    slot = np.empty(E, np.int64)
    slot[order] = base[tl[order], half[order]] * 128 + rank

    idx_arr = np.zeros((M, 2), object)
    s_arr = np.zeros((M, 2), object)
    for c in range(M):
        for s in range(2):
            npad_s = totpad[s] * 128
            ia = np.zeros(npad_s, np.int16)
            sel = (core == c) & (half == s)
            ia[slot[sel]] = idx16[sel].astype(np.int16)
            # host-built selection matrices: S[chunk, e, d] = (d==ld)*val
            sm = np.zeros((totpad[s], 128, 128), BF)
            sl = slot[sel]
            sm[sl // 128, sl % 128, ld[sel]] = val[sel].astype(BF)
            s_arr[c, s] = np.ascontiguousarray(
                sm.transpose(1, 0, 2).reshape(128, totpad[s] * 128)
            )
            # wrap indices for dma_gather: per call [16, 512] tiled x8 -> [128, 512]
            iw = ia.reshape(ncall[s], CH // 16, 16)
            iw = np.transpose(iw, (0, 2, 1))  # [ncall, 16, 512]
            iw = np.tile(iw, (1, 8, 1))  # [ncall, 128, 512]
            idx_arr[c, s] = np.concatenate(list(iw), axis=1)  # [128, ncall*512]

    x = np.asarray(x, np.float32)
    xT = np.zeros((M, D, NPAD), np.float32)
    for c in range(M):
        xT[c, :, :NPC] = x[c * NPC : (c + 1) * NPC].T

    conv_w = np.asarray(conv_w, np.float32)
    wtop = np.zeros((L, H, K * H), BF)
    wbot = np.zeros((L, H, K * H), BF)
    for l in range(L):
        for k in range(K):
            wtop[l, :, k * H : (k + 1) * H] = conv_w[l, k, :H].astype(BF)
            wbot[l, :, k * H : (k + 1) * H] = conv_w[l, k, H:].astype(BF)
    env_w = np.asarray(env_w, np.float32)
    env_b = np.asarray(env_b, np.float32)
    prep = dict(
        NCH=NCH,
        base=base,
        ncall=ncall,
        totpad=totpad,
        idx_arr=idx_arr,
        s_arr=s_arr,
        xT=xT,
        fc0_w=np.asarray(fc0_w, np.float32),
        b0=np.asarray(fc0_b, np.float32),
        wtop=wtop,
        wbot=wbot,
        env_w_bf=env_w.astype(BF),
        envb=env_b.reshape(L, 1, K).copy(),
        fc1_w=np.asarray(fc1_w, np.float32),
        b1_bcast=np.tile(np.asarray(fc1_b, np.float32), (128, 1)),
    )
    return prep


def _emulate(prep):
    """Numpy mirror of the device program (validates schedule/indexing)."""
    NCH, base, ncall, totpad = (
        prep["NCH"],
        prep["base"],
        prep["ncall"],
        prep["totpad"],
    )
    h_node = np.zeros((M, NPAD, H), np.float32)
    for c in range(M):
        z = prep["xT"][c].T @ prep["fc0_w"] + prep["b0"]
        h_node[c] = np.maximum(z, 0.0)

    def build_tables(h_node):
        lo = np.concatenate([h_node[c, :RA].astype(BF) for c in range(M)])
        hi = np.concatenate([h_node[c, RA:].astype(BF) for c in range(M)])
        return [lo, hi]

    tabs = build_tables(h_node)

    for l in range(L):
        new_h = np.zeros_like(h_node)
        for c in range(M):
            G = [None, None]
            for s in range(2):
                ia = prep["idx_arr"][c, s]
                idxs = []
                for g in range(ncall[s]):
                    blkw = ia[:16, g * 512 : (g + 1) * 512]
                    idxs.append(blkw.T.reshape(-1))
                idxs = np.concatenate(idxs).astype(np.int64)
                G[s] = tabs[s][idxs].astype(np.float32)
            gcnT = np.zeros((T, H, 128), np.float32)
            for t in range(T):
                acc = np.zeros((H, 128), np.float32)
                for s in range(2):
                    for jc in range(NCH[t, s]):
                        ch = base[t, s] + jc
                        g = G[s][ch * 128 : (ch + 1) * 128]
                        S = (
                            prep["s_arr"][c, s][:, ch * 128 : (ch + 1) * 128]
                            .astype(np.float32)
                        )
                        acc += g.astype(BF).astype(np.float32).T @ S
                gcnT[t] = acc
            hT_bf = h_node[c].T.astype(BF)
            for t in range(T):
                sl = slice(t * 128, (t + 1) * 128)
                ht = hT_bf[:, sl].astype(np.float32)
                z = ht.T @ prep["env_w_bf"][l].astype(np.float32)
                e = np.exp(z + prep["envb"][l][0][None, :])
                e = e / e.sum(axis=1, keepdims=True)
                gt = gcnT[t].astype(BF).astype(np.float32)
                O = gt.T @ prep["wtop"][l].astype(np.float32) + ht.T @ prep[
                    "wbot"
                ][l].astype(np.float32)
                O = O.reshape(128, K, H)
                mixed = np.einsum("nk,nkh->nh", e, O)
                new_h[c, sl] = np.maximum(mixed + h_node[c, sl], 0.0)
        h_node = new_h
        tabs = build_tables(h_node)

    out = np.zeros((N, C), np.float32)
    for c in range(M):
        z = h_node[c] @ prep["fc1_w"] + prep["b1_bcast"][0][None, :]
        out[c * NPC : (c + 1) * NPC] = z[:NPC]
    return out


def _build_program(prep):
    NCH, base, ncall, totpad = (
        prep["NCH"],
        prep["base"],
        prep["ncall"],
        prep["totpad"],
    )
    nc = bacc.Bacc(
        "TRN2", target_bir_lowering=False, debug=False, num_devices=M
    )
    # I/O
    xT = nc.dram_tensor("xT", [D, NPAD], F32, kind="ExternalInput")
    idx_io = [
        nc.dram_tensor(f"idx{s}", [128, ncall[s] * (CH // 16)], I16, kind="ExternalInput")
        for s in range(2)
    ]
    s_io = [
        nc.dram_tensor(f"smat{s}", [128, totpad[s] * 128], BF16, kind="ExternalInput")
        for s in range(2)
    ]
    fc0_w = nc.dram_tensor("fc0_w", [D, H], F32, kind="ExternalInput")
    b0col = nc.dram_tensor("b0col", [H, 1], F32, kind="ExternalInput")
    wtop = nc.dram_tensor("wtop", [L, H, K * H], BF16, kind="ExternalInput")
    wbot = nc.dram_tensor("wbot", [L, H, K * H], BF16, kind="ExternalInput")
    env_w = nc.dram_tensor("env_w", [L, H, K], BF16, kind="ExternalInput")
    envb = nc.dram_tensor("envb", [L, 1, K], F32, kind="ExternalInput")
    fc1_w = nc.dram_tensor("fc1_w", [H, C], F32, kind="ExternalInput")
    b1 = nc.dram_tensor("b1", [128, C], F32, kind="ExternalInput")
    out_io = nc.dram_tensor("out", [NPAD, C], F32, kind="ExternalOutput")

    # internal DRAM: per-layer lo/hi gather tables + AllGather inputs
    tab_lo = [
        nc.dram_tensor(f"tlo{l}", [LOROW, H], BF16, kind="Internal", addr_space="Shared")
        for l in range(L)
    ]
    tab_hi = [
        nc.dram_tensor(f"thi{l}", [HIROW, H], BF16, kind="Internal", addr_space="Shared")
        for l in range(L)
    ]
    agin_a = [
        nc.dram_tensor(f"aga{l}", [RA, H], BF16, kind="Internal") for l in range(L)
    ]
    agin_b = [
        nc.dram_tensor(f"agb{l}", [RB, H], BF16, kind="Internal") for l in range(L)
    ]

    RG = [list(range(M))]

    def ag(l, which):
        if which == 0:
            nc.gpsimd.collective_compute(
                "AllGather", mybir.AluOpType.bypass, replica_groups=RG,
                ins=[agin_a[l][:]], outs=[tab_lo[l][:]],
            )
        else:
            nc.gpsimd.collective_compute(
                "AllGather", mybir.AluOpType.bypass, replica_groups=RG,
                ins=[agin_b[l][:]], outs=[tab_hi[l][:]],
            )

    # gather call issue order: lo leads by 2 calls so hi-region collectives
    # have time to land while lo gathers run
    call_order = []
    ig = [0, 0]
    while ig[0] < ncall[0] or ig[1] < ncall[1]:
        for s in range(2):
            want = ig[0] <= ig[1] + 2 if s == 0 else ig[1] < ig[0] - 1
            if ig[s] < ncall[s] and (want or ig[1 - s] >= ncall[1 - s]):
                call_order.append((s, ig[s]))
                ig[s] += 1

    with tile.TileContext(nc) as tc:
        with tc.tile_pool(name="const", bufs=1) as const:
            ident = const.tile([128, 128], F32)
            make_identity(nc, ident[:])
            fc0w_sb = const.tile([D, H], F32)
            nc.sync.dma_start(fc0w_sb[:], fc0_w[:])
            b0_sb = const.tile([H, 1], F32)
            nc.sync.dma_start(b0_sb[:], b0col[:])
            wtop_sb = [const.tile([H, K * H], BF16, tag=f"wtop{l}", name=f"wtop{l}") for l in range(L)]
            wbot_sb = [const.tile([H, K * H], BF16, tag=f"wbot{l}", name=f"wbot{l}") for l in range(L)]
            envw_sb = [const.tile([H, K], BF16, tag=f"envw{l}", name=f"envw{l}") for l in range(L)]
            envb_sb = [const.tile([1, K], F32, tag=f"envb{l}", name=f"envb{l}") for l in range(L)]
            ones_sb = const.tile([1, 128], F32)
            nc.vector.memset(ones_sb[:], 1.0)
            for l in range(L):
                nc.sync.dma_start(wtop_sb[l][:], wtop[l])
                nc.sync.dma_start(wbot_sb[l][:], wbot[l])
                nc.sync.dma_start(envw_sb[l][:], env_w[l])
                nc.sync.dma_start(envb_sb[l][:], envb[l])
            fc1w_sb = const.tile([H, C], F32)
            nc.sync.dma_start(fc1w_sb[:], fc1_w[:])
            b1_sb = const.tile([128, C], F32)
            nc.sync.dma_start(b1_sb[:], b1[:])
            hT_bf = const.tile([H, NPAD], BF16)  # feat-major h (matmul operand)
            h_node = const.tile([128, T * 128], F32)  # node-major h blocks
            gcn_all = const.tile([H, T * 128], BF16)  # aggregated gcn per tile

            # ---------------- fc0 ----------------
            with (
                tc.tile_pool(name="fc0sb", bufs=3) as sb,
                tc.tile_pool(name="fc0ps", bufs=3, space="PSUM") as ps,
            ):
                for t in range(T):
                    xt = sb.tile([D, 128], F32, tag="xt")
                    nc.sync.dma_start(xt[:], xT[:, t * 128 : (t + 1) * 128])
                    z = ps.tile([H, 128], F32, tag="z")
                    nc.tensor.matmul(z[:], fc0w_sb[:], xt[:], start=True, stop=True)
                    h0t = sb.tile([H, 128], F32, tag="h0t")
                    nc.scalar.activation(
                        h0t[:], z[:], mybir.ActivationFunctionType.Relu, bias=b0_sb[:, 0:1]
                    )
                    nc.vector.tensor_copy(hT_bf[:, t * 128 : (t + 1) * 128], h0t[:])
                    ztr = ps.tile([128, H], F32, tag="ztr")
                    nc.tensor.transpose(ztr[:], h0t[:], ident[:])
                    nc.vector.tensor_copy(h_node[:, t * 128 : (t + 1) * 128], ztr[:])
                    hnb = sb.tile([128, H], BF16, tag="hnb")
                    nc.scalar.activation(
                        hnb[:], ztr[:], mybir.ActivationFunctionType.Copy
                    )
                    if t < TA:
                        nc.sync.dma_start(agin_a[0][t * 128 : (t + 1) * 128, :], hnb[:])
                    else:
                        o = (t - TA) * 128
                        nc.sync.dma_start(agin_b[0][o : o + 128, :], hnb[:])
                    if t == TA - 1:
                        ag(0, 0)
                if True:
                    ag(0, 1)

            # ---------------- conv layers ----------------
            for l in range(L):
                last = l == L - 1
                with (
                    tc.tile_pool(name=f"gsb{l}", bufs=2) as gp,
                    tc.tile_pool(name=f"ssb{l}", bufs=2) as sp,
                    tc.tile_pool(name=f"isb{l}", bufs=2) as ip,
                    tc.tile_pool(name=f"wsb{l}", bufs=4) as sb,
                    tc.tile_pool(name=f"ps{l}", bufs=1, space="PSUM") as ps1,
                    tc.tile_pool(name=f"ps2{l}", bufs=2, space="PSUM") as ps2,
                ):
                    gtiles = [[None] * ncall[0], [None] * ncall[1]]
                    stiles = [[None] * ncall[0], [None] * ncall[1]]
                    for s, g in call_order:
                        st = sp.tile([128, BLK, 128], BF16, tag=f"S{s}")
                        nc.sync.dma_start(
                            st[:], s_io[s][:, g * BLK * 128 : (g + 1) * BLK * 128]
                        )
                        stiles[s][g] = st
                        it = ip.tile([128, CH // 16], I16, tag=f"I{s}")
                        nc.sync.dma_start(
                            it[:], idx_io[s][:, g * (CH // 16) : (g + 1) * (CH // 16)]
                        )
                        gt = gp.tile([128, BLK, H], BF16, tag=f"G{s}")
                        src = (tab_lo[l] if s == 0 else tab_hi[l])[:, :]
                        nc.gpsimd.dma_gather(
                            gt[:],
                            src,
                            it[:],
                            num_idxs=CH,
                            num_idxs_reg=CH,
                            elem_size=H,
                            single_packet=(CH <= 1024),
                        )
                        gtiles[s][g] = gt

                    for t in range(T):
                        chunks = []
                        for s in range(2):
                            for j in range(NCH[t, s]):
                                chunks.append((s, int(base[t, s]) + j))
                        pg = ps2.tile([H, 128], F32, tag="gcn")
                        for j, (s, ch) in enumerate(chunks):
                            gt = gtiles[s][ch // BLK]
                            st = stiles[s][ch // BLK]
                            nc.tensor.matmul(
                                pg[:],
                                gt[:, ch % BLK, :],
                                st[:, ch % BLK, :],
                                start=(j == 0),
                                stop=(j == len(chunks) - 1),
                            )
                        nc.vector.tensor_copy(
                            gcn_all[:, t * 128 : (t + 1) * 128], pg[:]
                        )

                    for t in range(T):
                        hsl = hT_bf[:, t * 128 : (t + 1) * 128]
                        po = ps2.tile([128, K * H], F32, tag="O")
                        nc.tensor.matmul(
                            po[:],
                            gcn_all[:, t * 128 : (t + 1) * 128],
                            wtop_sb[l][:],
                            start=True,
                            stop=False,
                        )
                        nc.tensor.matmul(
                            po[:], hsl, wbot_sb[l][:], start=False, stop=True
                        )
                        pe = ps1.tile([128, K], F32, tag="e")
                        nc.tensor.matmul(pe[:], ones_sb[:], envb_sb[l][:], start=True, stop=False)
                        nc.tensor.matmul(pe[:], hsl, envw_sb[l][:], start=False, stop=True)
                        e_sb = sb.tile([128, K], F32, tag="e_sb")
                        nc.scalar.activation(
                            e_sb[:], pe[:], mybir.ActivationFunctionType.Exp
                        )
                        esum = sb.tile([128, 1], F32, tag="esum")
                        nc.vector.reduce_sum(esum[:], e_sb[:], axis=mybir.AxisListType.X)
                        nc.vector.reciprocal(esum[:], esum[:])

                        mixs = [sb.tile([128, H], F32, tag=f"mix{i}", name=f"mix{i}") for i in range(4)]
                        for k in range(K):
                            nc.scalar.activation(
                                mixs[k][:],
                                po[:, k * H : (k + 1) * H],
                                mybir.ActivationFunctionType.Copy,
                                scale=e_sb[:, k : k + 1],
                            )
                        nc.vector.tensor_add(mixs[0][:], mixs[0][:], mixs[1][:])
                        nc.vector.tensor_add(mixs[2][:], mixs[2][:], mixs[3][:])
                        nc.vector.tensor_add(mixs[0][:], mixs[0][:], mixs[2][:])
                        nc.scalar.activation(
                            mixs[1][:], mixs[0][:],
                            mybir.ActivationFunctionType.Copy,
                            scale=esum[:, 0:1],
                        )
                        hn = h_node[:, t * 128 : (t + 1) * 128]
                        nc.vector.tensor_add(mixs[1][:], mixs[1][:], hn)
                        nc.scalar.activation(
                            hn, mixs[1][:], mybir.ActivationFunctionType.Relu
                        )
                        ptr = ps1.tile([128, H], F32, tag="tr")
                        nc.tensor.transpose(ptr[:], hn, ident[:])
                        if not last:
                            nc.vector.tensor_copy(
                                hT_bf[:, t * 128 : (t + 1) * 128], ptr[:]
                            )
                            hnb = sb.tile([128, H], BF16, tag="hnb")
                            nc.scalar.activation(
                                hnb[:], hn, mybir.ActivationFunctionType.Copy
                            )
                            if t < TA:
                                nc.sync.dma_start(
                                    agin_a[1][t * 128 : (t + 1) * 128, :], hnb[:]
                                )
                            else:
                                o = (t - TA) * 128
                                nc.sync.dma_start(agin_b[1][o : o + 128, :], hnb[:])
                            if t == TA - 1:
                                ag(1, 0)
                            if t == T - 1:
                                ag(1, 1)
                        else:
                            h2T = sb.tile([H, 128], F32, tag="h2T")
                            nc.vector.tensor_copy(h2T[:], ptr[:])
                            pc = ps1.tile([128, C], F32, tag="c")
                            nc.tensor.matmul(
                                pc[:], h2T[:], fc1w_sb[:], start=True, stop=True
                            )
                            ob = sb.tile([128, C], F32, tag="ob")
                            nc.vector.tensor_add(ob[:], pc[:], b1_sb[:])
                            nc.sync.dma_start(
                                out_io[t * 128 : (t + 1) * 128, :], ob[:]
                            )
    nc.compile()
    return nc


def _in_maps(prep):
    maps = []
    for c in range(M):
        m = {
            "xT": prep["xT"][c],
            "fc0_w": prep["fc0_w"],
            "b0col": prep["b0"][:, None].copy(),
            "wtop": prep["wtop"],
            "wbot": prep["wbot"],
            "env_w": prep["env_w_bf"],
            "envb": prep["envb"].astype(np.float32),
            "fc1_w": prep["fc1_w"],
            "b1": prep["b1_bcast"],
        }
        for s in range(2):
            m[f"idx{s}"] = prep["idx_arr"][c, s]
            m[f"smat{s}"] = prep["s_arr"][c, s]
        maps.append(m)
    return maps


_compiled = {}


def _get_compiled(prep, key):
    if key not in _compiled:
        _compiled[key] = _build_program(prep)
    return _compiled[key]


def kernel(trace=False, **inputs):
    inputs = {k: np.asarray(v) for k, v in inputs.items()}
    prep = _preprocess(**inputs)
    key = hash(inputs["edge_index"].tobytes()) ^ hash(inputs["x"].tobytes()[:4096])
    nc = _get_compiled(prep, key)
    res = bass_utils.run_bass_kernel_spmd(
        nc, _in_maps(prep), core_ids=list(range(M)), trace=trace
    )
    out = np.zeros((N, C), np.float32)
    for c in range(M):
        out[c * NPC : (c + 1) * NPC] = res.results[c]["out"][:NPC]
    kernel.last_exec_time_ns = res.exec_time_ns
    kernel.last_results = res
    return out


# revision 13
# speedup vs baseline: 1.3458x; 1.3458x over previous
"""CaNet (moe_routing GNN) forward on 8 Trainium2 NeuronCores.

Sharding: nodes are range-partitioned across the 8 cores (6250 each, padded
to 6272 = 49*128). Each core owns the edges whose *destination* lands in its
node range. The GCN aggregation out[col] += val * h[row] is computed as a
sequence of tiny one-hot matmuls on the TensorEngine:

  - edges are sorted by destination tile (groups of 128 dest nodes) on the
    host and padded to multiples of 128 ("chunks");
  - the source features h[row] for one chunk are fetched from a replicated
    node-major bf16 table in HBM with the GpSimd dma_gather custom op
    (int16 indices; the table is split into a 24576-row "lo" region, the
    first 24 tiles of every core, and a 25600-row "hi" region, the last 25
    tiles of every core -- every chunk draws from one region);
  - the [128e x 128d] selection matrices S (S[e,d] = (d == ldest[e]) * val[e])
    are precomputed on the host (the edge list is a compile-time constant)
    and streamed from HBM with plain HWDGE DMA, one slab per gather call --
    this keeps the DVE out of the aggregation entirely;
  - psum_gcnT[f,d] += G_chunk.T @ S accumulates over the tile's chunks.

The lo/hi table split doubles as a latency hider: the AllGather between
layers is split into two collectives (tiles 0-23 -> tab_lo, tiles 24-48 ->
tab_hi), so the lo-region collective completes while the previous layer's
gathers still run and the next layer's lo gathers start immediately.

Dense per-node work (expert gate softmax, the K=4 expert convs, mixing,
residual relu, fc0/fc1) runs in bf16 matmuls + f32 psum per 128-node tile.

The per-core *program* is identical (SPMD); all per-core variation (gather
indices, S slabs, x slab) arrives via ExternalInputs. Chunk counts per
(tile, half) are max'd across cores so the schedule is static; padding
slots use idx=0 with an all-zero S row and contribute nothing.
"""

import sys

sys.path.insert(0, "/opt/trn_rl_repo")

import numpy as np
import ml_dtypes

import concourse.bacc as bacc
import concourse.tile as tile
import concourse.mybir as mybir
import concourse.bass as bass
from concourse import bass_utils
from concourse.masks import make_identity

# Problem constants (hardcoded per contract).
N = 50000
E = 800000
D = 128  # input dim
H = 128  # hidden dim
C = 47  # classes
K = 4  # experts
L = 2  # conv layers
M = 8  # cores

NPC = N // M  # 6250 nodes per core
T = (NPC + 127) // 128  # 49 tiles per core
NPAD = T * 128  # 6272
TA = 24  # tiles in the "lo" table region per core
TB = T - TA  # 25 tiles in the "hi" region
RA = TA * 128  # 3072 lo rows per core
RB = TB * 128  # 3200 hi rows per core
LOROW = M * RA  # 24576 lo region rows
HIROW = M * RB  # 25600 hi region rows
CH = 4096  # gather indices per dma_gather call
BLK = CH // 128  # 32 chunk blocks per gather call

F32 = mybir.dt.float32
BF16 = mybir.dt.bfloat16
I16 = mybir.dt.int16
BF = ml_dtypes.bfloat16


def _preprocess(x, edge_index, fc0_w, fc0_b, fc1_w, fc1_b, env_w, env_b, conv_w):
    """Host-side: degree/value computation, edge sort, static chunk schedule,
    per-core gather index + S-matrix arrays, weight packing."""
    row = np.asarray(edge_index[0], np.int64)
    col = np.asarray(edge_index[1], np.int64)

    deg = np.bincount(col, minlength=N).astype(np.float32)
    dinv = np.where(deg > 0, 1.0 / np.sqrt(deg), 0.0).astype(np.float32)
    val = (dinv[col] * dinv[row]).astype(np.float32)

    core = col // NPC
    dloc = col % NPC
    tl = dloc // 128
    ld = dloc % 128
    # table row: lo region holds tiles 0..TA-1 of every core, hi the rest
    score = row // NPC
    sloc = row % NPC
    half = (sloc >= RA).astype(np.int64)
    srow = np.where(half == 0, score * RA + sloc, score * RB + (sloc - RA))
    idx16 = srow  # already region-local

    cnt = np.bincount((core * T + tl) * 2 + half, minlength=M * T * 2).reshape(
        M, T, 2
    )
    nch = -(-cnt // 128)  # ceil div, [M, T, 2]
    NCH = nch.max(axis=0)  # [T, 2] static schedule
    assert NCH.sum() > 0
    tot = NCH.sum(axis=0)  # [2] total chunks per stream
    ncall = [int(-(-int(tot[s]) * 128 // CH)) for s in range(2)]
    totpad = [ncall[s] * BLK for s in range(2)]  # chunks incl. call padding

    base = np.zeros((T, 2), np.int64)
    base[1:] = NCH[:-1].cumsum(axis=0)

    gkey = (core * 2 + half) * T + tl
    order = np.argsort(gkey, kind="stable")
    gsort = gkey[order]
    starts = np.searchsorted(gsort, np.arange(M * 2 * T))
    rank = np.arange(E, dtype=np.int64) - starts[gsort]
    slot = np.empty(E, np.int64)
    slot[order] = base[tl[order], half[order]] * 128 + rank

    idx_arr = np.zeros((M, 2), object)
    s_arr = np.zeros((M, 2), object)
    for c in range(M):
        for s in range(2):
            npad_s = totpad[s] * 128
            ia = np.zeros(npad_s, np.int16)
            sel = (core == c) & (half == s)
            ia[slot[sel]] = idx16[sel].astype(np.int16)
            # host-built selection matrices: S[chunk, e, d] = (d==ld)*val
            sm = np.zeros((totpad[s], 128, 128), BF)
            sl = slot[sel]
            sm[sl // 128, sl % 128, ld[sel]] = val[sel].astype(BF)
            s_arr[c, s] = np.ascontiguousarray(
                sm.transpose(1, 0, 2).reshape(128, totpad[s] * 128)
            )
            # wrap indices for dma_gather: per call [16, 512] tiled x8 -> [128, 512]
            iw = ia.reshape(ncall[s], CH // 16, 16)
            iw = np.transpose(iw, (0, 2, 1))  # [ncall, 16, 512]
            iw = np.tile(iw, (1, 8, 1))  # [ncall, 128, 512]
            idx_arr[c, s] = np.concatenate(list(iw), axis=1)  # [128, ncall*512]

    x = np.asarray(x, np.float32)
    xT = np.zeros((M, D, NPAD), np.float32)
    for c in range(M):
        xT[c, :, :NPC] = x[c * NPC : (c + 1) * NPC].T

    conv_w = np.asarray(conv_w, np.float32)
    wtop = np.zeros((L, H, K * H), BF)
    wbot = np.zeros((L, H, K * H), BF)
    for l in range(L):
        for k in range(K):
            wtop[l, :, k * H : (k + 1) * H] = conv_w[l, k, :H].astype(BF)
            wbot[l, :, k * H : (k + 1) * H] = conv_w[l, k, H:].astype(BF)
    env_w = np.asarray(env_w, np.float32)
    env_b = np.asarray(env_b, np.float32)
    prep = dict(
        NCH=NCH,
        base=base,
        ncall=ncall,
        totpad=totpad,
        idx_arr=idx_arr,
        s_arr=s_arr,
        xT=xT,
        fc0_w=np.asarray(fc0_w, np.float32),
        b0=np.asarray(fc0_b, np.float32),
        wtop=wtop,
        wbot=wbot,
        env_w_bf=env_w.astype(BF),
        envb=env_b.reshape(L, 1, K).copy(),
        fc1_w=np.asarray(fc1_w, np.float32),
        b1_bcast=np.tile(np.asarray(fc1_b, np.float32), (128, 1)),
    )
    return prep


def _emulate(prep):
    """Numpy mirror of the device program (validates schedule/indexing)."""
    NCH, base, ncall, totpad = (
        prep["NCH"],
        prep["base"],
        prep["ncall"],
        prep["totpad"],
    )
    h_node = np.zeros((M, NPAD, H), np.float32)
    for c in range(M):
        z = prep["xT"][c].T @ prep["fc0_w"] + prep["b0"]
        h_node[c] = np.maximum(z, 0.0)

    def build_tables(h_node):
        lo = np.concatenate([h_node[c, :RA].astype(BF) for c in range(M)])
        hi = np.concatenate([h_node[c, RA:].astype(BF) for c in range(M)])
        return [lo, hi]

    tabs = build_tables(h_node)

    for l in range(L):
        new_h = np.zeros_like(h_node)
        for c in range(M):
            G = [None, None]
            for s in range(2):
                ia = prep["idx_arr"][c, s]
                idxs = []
                for g in range(ncall[s]):
                    blkw = ia[:16, g * 512 : (g + 1) * 512]
                    idxs.append(blkw.T.reshape(-1))
                idxs = np.concatenate(idxs).astype(np.int64)
                G[s] = tabs[s][idxs].astype(np.float32)
            gcnT = np.zeros((T, H, 128), np.float32)
            for t in range(T):
                acc = np.zeros((H, 128), np.float32)
                for s in range(2):
                    for jc in range(NCH[t, s]):
                        ch = base[t, s] + jc
                        g = G[s][ch * 128 : (ch + 1) * 128]
                        S = (
                            prep["s_arr"][c, s][:, ch * 128 : (ch + 1) * 128]
                            .astype(np.float32)
                        )
                        acc += g.astype(BF).astype(np.float32).T @ S
                gcnT[t] = acc
            hT_bf = h_node[c].T.astype(BF)
            for t in range(T):
                sl = slice(t * 128, (t + 1) * 128)
                ht = hT_bf[:, sl].astype(np.float32)
                z = ht.T @ prep["env_w_bf"][l].astype(np.float32)
                e = np.exp(z + prep["envb"][l][0][None, :])
                e = e / e.sum(axis=1, keepdims=True)
                gt = gcnT[t].astype(BF).astype(np.float32)
                O = gt.T @ prep["wtop"][l].astype(np.float32) + ht.T @ prep[
                    "wbot"
                ][l].astype(np.float32)
                O = O.reshape(128, K, H)
                mixed = np.einsum("nk,nkh->nh", e, O)
                new_h[c, sl] = np.maximum(mixed + h_node[c, sl], 0.0)
        h_node = new_h
        tabs = build_tables(h_node)

    out = np.zeros((N, C), np.float32)
    for c in range(M):
        z = h_node[c] @ prep["fc1_w"] + prep["b1_bcast"][0][None, :]
        out[c * NPC : (c + 1) * NPC] = z[:NPC]
    return out


def _build_program(prep):
    NCH, base, ncall, totpad = (
        prep["NCH"],
        prep["base"],
        prep["ncall"],
        prep["totpad"],
    )
    nc = bacc.Bacc(
        "TRN2", target_bir_lowering=False, debug=False, num_devices=M
    )
    # I/O
    xT = nc.dram_tensor("xT", [D, NPAD], F32, kind="ExternalInput")
    idx_io = [
        nc.dram_tensor(f"idx{s}", [128, ncall[s] * (CH // 16)], I16, kind="ExternalInput")
        for s in range(2)
    ]
    s_io = [
        nc.dram_tensor(f"smat{s}", [128, totpad[s] * 128], BF16, kind="ExternalInput")
        for s in range(2)
    ]
    fc0_w = nc.dram_tensor("fc0_w", [D, H], F32, kind="ExternalInput")
    b0col = nc.dram_tensor("b0col", [H, 1], F32, kind="ExternalInput")
    wtop = nc.dram_tensor("wtop", [L, H, K * H], BF16, kind="ExternalInput")
    wbot = nc.dram_tensor("wbot", [L, H, K * H], BF16, kind="ExternalInput")
    env_w = nc.dram_tensor("env_w", [L, H, K], BF16, kind="ExternalInput")
    envb = nc.dram_tensor("envb", [L, 1, K], F32, kind="ExternalInput")
    fc1_w = nc.dram_tensor("fc1_w", [H, C], F32, kind="ExternalInput")
    b1 = nc.dram_tensor("b1", [128, C], F32, kind="ExternalInput")
    out_io = nc.dram_tensor("out", [NPAD, C], F32, kind="ExternalOutput")

    # internal DRAM: per-layer lo/hi gather tables + AllGather inputs
    tab_lo = [
        nc.dram_tensor(f"tlo{l}", [LOROW, H], BF16, kind="Internal", addr_space="Shared")
        for l in range(L)
    ]
    tab_hi = [
        nc.dram_tensor(f"thi{l}", [HIROW, H], BF16, kind="Internal", addr_space="Shared")
        for l in range(L)
    ]
    agin_a = [
        nc.dram_tensor(f"aga{l}", [RA, H], BF16, kind="Internal") for l in range(L)
    ]
    agin_b = [
        nc.dram_tensor(f"agb{l}", [RB, H], BF16, kind="Internal") for l in range(L)
    ]

    RG = [list(range(M))]

    def ag(l, which):
        if which == 0:
            nc.gpsimd.collective_compute(
                "AllGather", mybir.AluOpType.bypass, replica_groups=RG,
                ins=[agin_a[l][:]], outs=[tab_lo[l][:]],
            )
        else:
            nc.gpsimd.collective_compute(
                "AllGather", mybir.AluOpType.bypass, replica_groups=RG,
                ins=[agin_b[l][:]], outs=[tab_hi[l][:]],
            )

    # gather call issue order: lo leads by 2 calls so hi-region collectives
    # have time to land while lo gathers run
    call_order = []
    ig = [0, 0]
    while ig[0] < ncall[0] or ig[1] < ncall[1]:
        for s in range(2):
            want = ig[0] <= ig[1] + 2 if s == 0 else ig[1] < ig[0] - 1
            if ig[s] < ncall[s] and (want or ig[1 - s] >= ncall[1 - s]):
                call_order.append((s, ig[s]))
                ig[s] += 1

    with tile.TileContext(nc) as tc:
        with tc.tile_pool(name="const", bufs=1) as const:
            ident = const.tile([128, 128], F32)
            make_identity(nc, ident[:])
            fc0w_sb = const.tile([D, H], F32)
            nc.sync.dma_start(fc0w_sb[:], fc0_w[:])
            b0_sb = const.tile([H, 1], F32)
            nc.sync.dma_start(b0_sb[:], b0col[:])
            wtop_sb = [const.tile([H, K * H], BF16, tag=f"wtop{l}", name=f"wtop{l}") for l in range(L)]
            wbot_sb = [const.tile([H, K * H], BF16, tag=f"wbot{l}", name=f"wbot{l}") for l in range(L)]
            envw_sb = [const.tile([H, K], BF16, tag=f"envw{l}", name=f"envw{l}") for l in range(L)]
            envb_sb = [const.tile([1, K], F32, tag=f"envb{l}", name=f"envb{l}") for l in range(L)]
            ones_sb = const.tile([1, 128], F32)
            nc.vector.memset(ones_sb[:], 1.0)
            for l in range(L):
                nc.sync.dma_start(wtop_sb[l][:], wtop[l])
                nc.sync.dma_start(wbot_sb[l][:], wbot[l])
                nc.sync.dma_start(envw_sb[l][:], env_w[l])
                nc.sync.dma_start(envb_sb[l][:], envb[l])
            fc1w_sb = const.tile([H, C], F32)
            nc.sync.dma_start(fc1w_sb[:], fc1_w[:])
            b1_sb = const.tile([128, C], F32)
            nc.sync.dma_start(b1_sb[:], b1[:])
            idx_sb = [
                const.tile([128, ncall[s] * (CH // 16)], I16, tag=f"idx{s}", name=f"idxsb{s}")
                for s in range(2)
            ]
            for s in range(2):
                nc.sync.dma_start(idx_sb[s][:], idx_io[s][:])
            hT_bf = const.tile([H, NPAD], BF16)  # feat-major h (matmul operand)
            h_node = const.tile([128, T * 128], F32)  # node-major h blocks
            gcn_all = const.tile([H, T * 128], BF16)  # aggregated gcn per tile

            # ---------------- fc0 ----------------
            with (
                tc.tile_pool(name="fc0sb", bufs=3) as sb,
                tc.tile_pool(name="fc0ps", bufs=3, space="PSUM") as ps,
            ):
                for t in range(T):
                    xt = sb.tile([D, 128], F32, tag="xt")
                    nc.sync.dma_start(xt[:], xT[:, t * 128 : (t + 1) * 128])
                    z = ps.tile([H, 128], F32, tag="z")
                    nc.tensor.matmul(z[:], fc0w_sb[:], xt[:], start=True, stop=True)
                    h0t = sb.tile([H, 128], F32, tag="h0t")
                    nc.scalar.activation(
                        h0t[:], z[:], mybir.ActivationFunctionType.Relu, bias=b0_sb[:, 0:1]
                    )
                    nc.vector.tensor_copy(hT_bf[:, t * 128 : (t + 1) * 128], h0t[:])
                    ztr = ps.tile([128, H], F32, tag="ztr")
                    nc.tensor.transpose(ztr[:], h0t[:], ident[:])
                    nc.vector.tensor_copy(h_node[:, t * 128 : (t + 1) * 128], ztr[:])
                    hnb = sb.tile([128, H], BF16, tag="hnb")
                    nc.scalar.activation(
                        hnb[:], ztr[:], mybir.ActivationFunctionType.Copy
                    )
                    if t < TA:
                        nc.sync.dma_start(agin_a[0][t * 128 : (t + 1) * 128, :], hnb[:])
                    else:
                        o = (t - TA) * 128
                        nc.sync.dma_start(agin_b[0][o : o + 128, :], hnb[:])
                    if t == TA - 1:
                        ag(0, 0)
                if True:
                    ag(0, 1)

            # ---------------- conv layers ----------------
            for l in range(L):
                last = l == L - 1
                with (
                    tc.tile_pool(name=f"gsb{l}", bufs=3) as gp,
                    tc.tile_pool(name=f"ssb{l}", bufs=3) as sp,
                    tc.tile_pool(name=f"wsb{l}", bufs=4) as sb,
                    tc.tile_pool(name=f"ps{l}", bufs=1, space="PSUM") as ps1,
                    tc.tile_pool(name=f"ps2{l}", bufs=2, space="PSUM") as ps2,
                ):
                    gtiles = [[None] * ncall[0], [None] * ncall[1]]
                    stiles = [[None] * ncall[0], [None] * ncall[1]]
                    for s, g in call_order:
                        st = sp.tile([128, BLK, 128], BF16, tag=f"S{s}")
                        nc.sync.dma_start(
                            st[:], s_io[s][:, g * BLK * 128 : (g + 1) * BLK * 128]
                        )
                        stiles[s][g] = st
                        gt = gp.tile([128, BLK, H], BF16, tag=f"G{s}")
                        src = (tab_lo[l] if s == 0 else tab_hi[l])[:, :]
                        nc.gpsimd.dma_gather(
                            gt[:],
                            src,
                            idx_sb[s][:, g * (CH // 16) : (g + 1) * (CH // 16)],
                            num_idxs=CH,
                            num_idxs_reg=CH,
                            elem_size=H,
                            single_packet=(CH <= 1024),
                        )
                        gtiles[s][g] = gt

                    for t in range(T):
                        chunks = []
                        for s in range(2):
                            for j in range(NCH[t, s]):
                                chunks.append((s, int(base[t, s]) + j))
                        pg = ps2.tile([H, 128], F32, tag="gcn")
                        for j, (s, ch) in enumerate(chunks):
                            gt = gtiles[s][ch // BLK]
                            st = stiles[s][ch // BLK]
                            nc.tensor.matmul(
                                pg[:],
                                gt[:, ch % BLK, :],
                                st[:, ch % BLK, :],
                                start=(j == 0),
                                stop=(j == len(chunks) - 1),
                            )
                        nc.vector.tensor_copy(
                            gcn_all[:, t * 128 : (t + 1) * 128], pg[:]
                        )

                    for t in range(T):
                        hsl = hT_bf[:, t * 128 : (t + 1) * 128]
                        po = ps2.tile([128, K * H], F32, tag="O")
                        nc.tensor.matmul(
                            po[:],
                            gcn_all[:, t * 128 : (t + 1) * 128],
                            wtop_sb[l][:],
                            start=True,
                            stop=False,
                        )
                        nc.tensor.matmul(
                            po[:], hsl, wbot_sb[l][:], start=False, stop=True
                        )
                        pe = ps1.tile([128, K], F32, tag="e")
                        nc.tensor.matmul(pe[:], ones_sb[:], envb_sb[l][:], start=True, stop=False)
                        nc.tensor.matmul(pe[:], hsl, envw_sb[l][:], start=False, stop=True)
                        e_sb = sb.tile([128, K], F32, tag="e_sb")
                        nc.scalar.activation(
                            e_sb[:], pe[:], mybir.ActivationFunctionType.Exp
                        )
                        esum = sb.tile([128, 1], F32, tag="esum")
                        nc.vector.reduce_sum(esum[:], e_sb[:], axis=mybir.AxisListType.X)
                        nc.vector.reciprocal(esum[:], esum[:])

                        mixs = [sb.tile([128, H], F32, tag=f"mix{i}", name=f"mix{i}") for i in range(4)]
                        for k in range(K):
                            nc.scalar.activation(
                                mixs[k][:],
                                po[:, k * H : (k + 1) * H],
                                mybir.ActivationFunctionType.Copy,
                                scale=e_sb[:, k : k + 1],
                            )
                        nc.vector.tensor_add(mixs[0][:], mixs[0][:], mixs[1][:])
                        nc.vector.tensor_add(mixs[2][:], mixs[2][:], mixs[3][:])
                        nc.vector.tensor_add(mixs[0][:], mixs[0][:], mixs[2][:])
                        nc.scalar.activation(
                            mixs[1][:], mixs[0][:],
                            mybir.ActivationFunctionType.Copy,
                            scale=esum[:, 0:1],
                        )
                        hn = h_node[:, t * 128 : (t + 1) * 128]
                        nc.vector.tensor_add(mixs[1][:], mixs[1][:], hn)
                        nc.scalar.activation(
                            hn, mixs[1][:], mybir.ActivationFunctionType.Relu
                        )
                        ptr = ps1.tile([128, H], F32, tag="tr")
                        nc.tensor.transpose(ptr[:], hn, ident[:])
                        if not last:
                            nc.vector.tensor_copy(
                                hT_bf[:, t * 128 : (t + 1) * 128], ptr[:]
                            )
                            hnb = sb.tile([128, H], BF16, tag="hnb")
                            nc.scalar.activation(
                                hnb[:], hn, mybir.ActivationFunctionType.Copy
                            )
                            if t < TA:
                                nc.sync.dma_start(
                                    agin_a[1][t * 128 : (t + 1) * 128, :], hnb[:]
                                )
                            else:
                                o = (t - TA) * 128
                                nc.sync.dma_start(agin_b[1][o : o + 128, :], hnb[:])
                            if t == TA - 1:
                                ag(1, 0)
                            if t == T - 1:
                                ag(1, 1)
                        else:
                            h2T = sb.tile([H, 128], F32, tag="h2T")
                            nc.vector.tensor_copy(h2T[:], ptr[:])
                            pc = ps1.tile([128, C], F32, tag="c")
                            nc.tensor.matmul(
                                pc[:], h2T[:], fc1w_sb[:], start=True, stop=True
                            )
                            ob = sb.tile([128, C], F32, tag="ob")
                            nc.vector.tensor_add(ob[:], pc[:], b1_sb[:])
                            nc.sync.dma_start(
                                out_io[t * 128 : (t + 1) * 128, :], ob[:]
                            )
    nc.compile()
    return nc


def _in_maps(prep):
    maps = []
    for c in range(M):
        m = {
            "xT": prep["xT"][c],
            "fc0_w": prep["fc0_w"],
            "b0col": prep["b0"][:, None].copy(),
            "wtop": prep["wtop"],
            "wbot": prep["wbot"],
            "env_w": prep["env_w_bf"],
            "envb": prep["envb"].astype(np.float32),
            "fc1_w": prep["fc1_w"],
            "b1": prep["b1_bcast"],
        }
        for s in range(2):
            m[f"idx{s}"] = prep["idx_arr"][c, s]
            m[f"smat{s}"] = prep["s_arr"][c, s]
        maps.append(m)
    return maps


_compiled = {}


def _get_compiled(prep, key):
    if key not in _compiled:
        _compiled[key] = _build_program(prep)
    return _compiled[key]


def kernel(trace=False, **inputs):
    inputs = {k: np.asarray(v) for k, v in inputs.items()}
    prep = _preprocess(**inputs)
    key = hash(inputs["edge_index"].tobytes()) ^ hash(inputs["x"].tobytes()[:4096])
    nc = _get_compiled(prep, key)
    res = bass_utils.run_bass_kernel_spmd(
        nc, _in_maps(prep), core_ids=list(range(M)), trace=trace
    )
    out = np.zeros((N, C), np.float32)
    for c in range(M):
        out[c * NPC : (c + 1) * NPC] = res.results[c]["out"][:NPC]
    kernel.last_exec_time_ns = res.exec_time_ns
    kernel.last_results = res
    return out


# revision 14
# speedup vs baseline: 1.4373x; 1.0680x over previous
"""CaNet (moe_routing GNN) forward on 8 Trainium2 NeuronCores.

Sharding: nodes are range-partitioned across the 8 cores (6250 each, padded
to 6272 = 49*128). Each core owns the edges whose *destination* lands in its
node range. The GCN aggregation out[col] += val * h[row] is computed as a
sequence of tiny one-hot matmuls on the TensorEngine:

  - edges are sorted by destination tile (groups of 128 dest nodes) on the
    host and padded to multiples of 128 ("chunks");
  - the source features h[row] for one chunk are fetched from a replicated
    node-major bf16 table in HBM with the GpSimd dma_gather custom op
    (int16 indices; the table is split into a 24576-row "lo" region, the
    first 24 tiles of every core, and a 25600-row "hi" region, the last 25
    tiles of every core -- every chunk draws from one region);
  - the [128e x 128d] selection matrices S (S[e,d] = (d == ldest[e]) * val[e])
    are precomputed on the host (the edge list is a compile-time constant)
    and streamed from HBM with plain HWDGE DMA, one slab per gather call --
    this keeps the DVE out of the aggregation entirely;
  - psum_gcnT[f,d] += G_chunk.T @ S accumulates over the tile's chunks.

The lo/hi table split doubles as a latency hider: the AllGather between
layers is split into two collectives (tiles 0-23 -> tab_lo, tiles 24-48 ->
tab_hi), so the lo-region collective completes while the previous layer's
gathers still run and the next layer's lo gathers start immediately.

Dense per-node work (expert gate softmax, the K=4 expert convs, mixing,
residual relu, fc0/fc1) runs in bf16 matmuls + f32 psum per 128-node tile.

The per-core *program* is identical (SPMD); all per-core variation (gather
indices, S slabs, x slab) arrives via ExternalInputs. Chunk counts per
(tile, half) are max'd across cores so the schedule is static; padding
slots use idx=0 with an all-zero S row and contribute nothing.
"""

import sys

sys.path.insert(0, "/opt/trn_rl_repo")

import numpy as np
import ml_dtypes

import concourse.bacc as bacc
import concourse.tile as tile
import concourse.mybir as mybir
import concourse.bass as bass
from concourse import bass_utils
from concourse.masks import make_identity

# Problem constants (hardcoded per contract).
N = 50000
E = 800000
D = 128  # input dim
H = 128  # hidden dim
C = 47  # classes
K = 4  # experts
L = 2  # conv layers
M = 8  # cores

NPC = N // M  # 6250 nodes per core
T = (NPC + 127) // 128  # 49 tiles per core
NPAD = T * 128  # 6272
TA = 24  # tiles in the "lo" table region per core
TB = T - TA  # 25 tiles in the "hi" region
RA = TA * 128  # 3072 lo rows per core
RB = TB * 128  # 3200 hi rows per core
LOROW = M * RA  # 24576 lo region rows
HIROW = M * RB  # 25600 hi region rows
CH = 4096  # gather indices per dma_gather call
BLK = CH // 128  # 32 chunk blocks per gather call

F32 = mybir.dt.float32
BF16 = mybir.dt.bfloat16
I16 = mybir.dt.int16
BF = ml_dtypes.bfloat16


def _preprocess(x, edge_index, fc0_w, fc0_b, fc1_w, fc1_b, env_w, env_b, conv_w):
    """Host-side: degree/value computation, edge sort, static chunk schedule,
    per-core gather index + S-matrix arrays, weight packing."""
    row = np.asarray(edge_index[0], np.int64)
    col = np.asarray(edge_index[1], np.int64)

    deg = np.bincount(col, minlength=N).astype(np.float32)
    dinv = np.where(deg > 0, 1.0 / np.sqrt(deg), 0.0).astype(np.float32)
    val = (dinv[col] * dinv[row]).astype(np.float32)

    core = col // NPC
    dloc = col % NPC
    tl = dloc // 128
    ld = dloc % 128
    # table row: lo region holds tiles 0..TA-1 of every core, hi the rest
    score = row // NPC
    sloc = row % NPC
    half = (sloc >= RA).astype(np.int64)
    srow = np.where(half == 0, score * RA + sloc, score * RB + (sloc - RA))
    idx16 = srow  # already region-local

    cnt = np.bincount((core * T + tl) * 2 + half, minlength=M * T * 2).reshape(
        M, T, 2
    )
    nch = -(-cnt // 128)  # ceil div, [M, T, 2]
    NCH = nch.max(axis=0)  # [T, 2] static schedule
    assert NCH.sum() > 0
    tot = NCH.sum(axis=0)  # [2] total chunks per stream
    ncall = [int(-(-int(tot[s]) * 128 // CH)) for s in range(2)]
    totpad = [ncall[s] * BLK for s in range(2)]  # chunks incl. call padding

    base = np.zeros((T, 2), np.int64)
    base[1:] = NCH[:-1].cumsum(axis=0)

    gkey = (core * 2 + half) * T + tl
    order = np.argsort(gkey, kind="stable")
    gsort = gkey[order]
    starts = np.searchsorted(gsort, np.arange(M * 2 * T))
    rank = np.arange(E, dtype=np.int64) - starts[gsort]
    slot = np.empty(E, np.int64)
    slot[order] = base[tl[order], half[order]] * 128 + rank

    idx_arr = np.zeros((M, 2), object)
    s_arr = np.zeros((M, 2), object)
    for c in range(M):
        for s in range(2):
            npad_s = totpad[s] * 128
            ia = np.zeros(npad_s, np.int16)
            sel = (core == c) & (half == s)
            ia[slot[sel]] = idx16[sel].astype(np.int16)
            # host-built selection matrices: S[chunk, e, d] = (d==ld)*val
            sm = np.zeros((totpad[s], 128, 128), BF)
            sl = slot[sel]
            sm[sl // 128, sl % 128, ld[sel]] = val[sel].astype(BF)
            s_arr[c, s] = np.ascontiguousarray(
                sm.transpose(1, 0, 2).reshape(128, totpad[s] * 128)
            )
            # wrap indices for dma_gather: per call [16, 512] tiled x8 -> [128, 512]
            iw = ia.reshape(ncall[s], CH // 16, 16)
            iw = np.transpose(iw, (0, 2, 1))  # [ncall, 16, 512]
            iw = np.tile(iw, (1, 8, 1))  # [ncall, 128, 512]
            idx_arr[c, s] = np.concatenate(list(iw), axis=1)  # [128, ncall*512]

    x = np.asarray(x, np.float32)
    xT = np.zeros((M, D, NPAD), np.float32)
    for c in range(M):
        xT[c, :, :NPC] = x[c * NPC : (c + 1) * NPC].T

    conv_w = np.asarray(conv_w, np.float32)
    wtop = np.zeros((L, H, K * H), BF)
    wbot = np.zeros((L, H, K * H), BF)
    for l in range(L):
        for k in range(K):
            wtop[l, :, k * H : (k + 1) * H] = conv_w[l, k, :H].astype(BF)
            wbot[l, :, k * H : (k + 1) * H] = conv_w[l, k, H:].astype(BF)
    env_w = np.asarray(env_w, np.float32)
    env_b = np.asarray(env_b, np.float32)
    prep = dict(
        NCH=NCH,
        base=base,
        tot=tot,
        ncall=ncall,
        totpad=totpad,
        idx_arr=idx_arr,
        s_arr=s_arr,
        xT=xT,
        fc0_w=np.asarray(fc0_w, np.float32),
        b0=np.asarray(fc0_b, np.float32),
        wtop=wtop,
        wbot=wbot,
        env_w_bf=env_w.astype(BF),
        envb=env_b.reshape(L, 1, K).copy(),
        fc1_w=np.asarray(fc1_w, np.float32),
        b1_bcast=np.tile(np.asarray(fc1_b, np.float32), (128, 1)),
    )
    return prep


def _emulate(prep):
    """Numpy mirror of the device program (validates schedule/indexing)."""
    NCH, base, ncall, totpad = (
        prep["NCH"],
        prep["base"],
        prep["ncall"],
        prep["totpad"],
    )
    h_node = np.zeros((M, NPAD, H), np.float32)
    for c in range(M):
        z = prep["xT"][c].T @ prep["fc0_w"] + prep["b0"]
        h_node[c] = np.maximum(z, 0.0)

    def build_tables(h_node):
        lo = np.concatenate([h_node[c, :RA].astype(BF) for c in range(M)])
        hi = np.concatenate([h_node[c, RA:].astype(BF) for c in range(M)])
        return [lo, hi]

    tabs = build_tables(h_node)

    for l in range(L):
        new_h = np.zeros_like(h_node)
        for c in range(M):
            G = [None, None]
            for s in range(2):
                ia = prep["idx_arr"][c, s]
                idxs = []
                for g in range(ncall[s]):
                    blkw = ia[:16, g * 512 : (g + 1) * 512]
                    idxs.append(blkw.T.reshape(-1))
                idxs = np.concatenate(idxs).astype(np.int64)
                G[s] = tabs[s][idxs].astype(np.float32)
            gcnT = np.zeros((T, H, 128), np.float32)
            for t in range(T):
                acc = np.zeros((H, 128), np.float32)
                for s in range(2):
                    for jc in range(NCH[t, s]):
                        ch = base[t, s] + jc
                        g = G[s][ch * 128 : (ch + 1) * 128]
                        S = (
                            prep["s_arr"][c, s][:, ch * 128 : (ch + 1) * 128]
                            .astype(np.float32)
                        )
                        acc += g.astype(BF).astype(np.float32).T @ S
                gcnT[t] = acc
            hT_bf = h_node[c].T.astype(BF)
            for t in range(T):
                sl = slice(t * 128, (t + 1) * 128)
                ht = hT_bf[:, sl].astype(np.float32)
                z = ht.T @ prep["env_w_bf"][l].astype(np.float32)
                e = np.exp(z + prep["envb"][l][0][None, :])
                e = e / e.sum(axis=1, keepdims=True)
                gt = gcnT[t].astype(BF).astype(np.float32)
                O = gt.T @ prep["wtop"][l].astype(np.float32) + ht.T @ prep[
                    "wbot"
                ][l].astype(np.float32)
                O = O.reshape(128, K, H)
                mixed = np.einsum("nk,nkh->nh", e, O)
                new_h[c, sl] = np.maximum(mixed + h_node[c, sl], 0.0)
        h_node = new_h
        tabs = build_tables(h_node)

    out = np.zeros((N, C), np.float32)
    for c in range(M):
        z = h_node[c] @ prep["fc1_w"] + prep["b1_bcast"][0][None, :]
        out[c * NPC : (c + 1) * NPC] = z[:NPC]
    return out


def _build_program(prep):
    NCH, base, ncall, totpad = (
        prep["NCH"],
        prep["base"],
        prep["ncall"],
        prep["totpad"],
    )
    tot = prep["tot"]
    nc = bacc.Bacc(
        "TRN2", target_bir_lowering=False, debug=False, num_devices=M
    )
    # I/O
    xT = nc.dram_tensor("xT", [D, NPAD], F32, kind="ExternalInput")
    idx_io = [
        nc.dram_tensor(f"idx{s}", [128, ncall[s] * (CH // 16)], I16, kind="ExternalInput")
        for s in range(2)
    ]
    s_io = [
        nc.dram_tensor(f"smat{s}", [128, totpad[s] * 128], BF16, kind="ExternalInput")
        for s in range(2)
    ]
    fc0_w = nc.dram_tensor("fc0_w", [D, H], F32, kind="ExternalInput")
    b0col = nc.dram_tensor("b0col", [H, 1], F32, kind="ExternalInput")
    wtop = nc.dram_tensor("wtop", [L, H, K * H], BF16, kind="ExternalInput")
    wbot = nc.dram_tensor("wbot", [L, H, K * H], BF16, kind="ExternalInput")
    env_w = nc.dram_tensor("env_w", [L, H, K], BF16, kind="ExternalInput")
    envb = nc.dram_tensor("envb", [L, 1, K], F32, kind="ExternalInput")
    fc1_w = nc.dram_tensor("fc1_w", [H, C], F32, kind="ExternalInput")
    b1 = nc.dram_tensor("b1", [128, C], F32, kind="ExternalInput")
    out_io = nc.dram_tensor("out", [NPAD, C], F32, kind="ExternalOutput")

    # internal DRAM: per-layer lo/hi gather tables + AllGather inputs
    tab_lo = [
        nc.dram_tensor(f"tlo{l}", [LOROW, H], BF16, kind="Internal", addr_space="Shared")
        for l in range(L)
    ]
    tab_hi = [
        nc.dram_tensor(f"thi{l}", [HIROW, H], BF16, kind="Internal", addr_space="Shared")
        for l in range(L)
    ]
    agin_a = [
        nc.dram_tensor(f"aga{l}", [RA, H], BF16, kind="Internal") for l in range(L)
    ]
    agin_b = [
        nc.dram_tensor(f"agb{l}", [RB, H], BF16, kind="Internal") for l in range(L)
    ]

    RG = [list(range(M))]

    def ag(l, which):
        if which == 0:
            nc.gpsimd.collective_compute(
                "AllGather", mybir.AluOpType.bypass, replica_groups=RG,
                ins=[agin_a[l][:]], outs=[tab_lo[l][:]],
            )
        else:
            nc.gpsimd.collective_compute(
                "AllGather", mybir.AluOpType.bypass, replica_groups=RG,
                ins=[agin_b[l][:]], outs=[tab_hi[l][:]],
            )

    # gather call issue order: lo leads by 2 calls so hi-region collectives
    # have time to land while lo gathers run
    call_order = []
    ig = [0, 0]
    while ig[0] < ncall[0] or ig[1] < ncall[1]:
        for s in range(2):
            want = ig[0] <= ig[1] + 2 if s == 0 else ig[1] < ig[0] - 1
            if ig[s] < ncall[s] and (want or ig[1 - s] >= ncall[1 - s]):
                call_order.append((s, ig[s]))
                ig[s] += 1

    with tile.TileContext(nc) as tc:
        with tc.tile_pool(name="const", bufs=1) as const:
            ident = const.tile([128, 128], F32)
            make_identity(nc, ident[:])
            fc0w_sb = const.tile([D, H], F32)
            nc.sync.dma_start(fc0w_sb[:], fc0_w[:])
            b0_sb = const.tile([H, 1], F32)
            nc.sync.dma_start(b0_sb[:], b0col[:])
            wtop_sb = [const.tile([H, K * H], BF16, tag=f"wtop{l}", name=f"wtop{l}") for l in range(L)]
            wbot_sb = [const.tile([H, K * H], BF16, tag=f"wbot{l}", name=f"wbot{l}") for l in range(L)]
            envw_sb = [const.tile([H, K], BF16, tag=f"envw{l}", name=f"envw{l}") for l in range(L)]
            envb_sb = [const.tile([1, K], F32, tag=f"envb{l}", name=f"envb{l}") for l in range(L)]
            ones_sb = const.tile([1, 128], F32)
            nc.vector.memset(ones_sb[:], 1.0)
            for l in range(L):
                nc.sync.dma_start(wtop_sb[l][:], wtop[l])
                nc.sync.dma_start(wbot_sb[l][:], wbot[l])
                nc.sync.dma_start(envw_sb[l][:], env_w[l])
                nc.sync.dma_start(envb_sb[l][:], envb[l])
            fc1w_sb = const.tile([H, C], F32)
            nc.sync.dma_start(fc1w_sb[:], fc1_w[:])
            b1_sb = const.tile([128, C], F32)
            nc.sync.dma_start(b1_sb[:], b1[:])
            idx_sb = [
                const.tile([128, ncall[s] * (CH // 16)], I16, tag=f"idx{s}", name=f"idxsb{s}")
                for s in range(2)
            ]
            for s in range(2):
                nc.sync.dma_start(idx_sb[s][:], idx_io[s][:])
            hT_bf = const.tile([H, NPAD], BF16)  # feat-major h (matmul operand)
            h_node = const.tile([128, T * 128], F32)  # node-major h blocks
            gcn_all = const.tile([H, T * 128], BF16)  # aggregated gcn per tile

            # ---------------- fc0 ----------------
            with (
                tc.tile_pool(name="fc0sb", bufs=3) as sb,
                tc.tile_pool(name="fc0ps", bufs=3, space="PSUM") as ps,
            ):
                xt4s = {}
                for tb in range(0, T, 4):
                    nt = min(4, T - tb)
                    xt4 = sb.tile([D, 512], F32, tag="xt")
                    nc.sync.dma_start(
                        xt4[:, 0 : nt * 128],
                        xT[:, tb * 128 : (tb + nt) * 128],
                    )
                    xt4s[tb] = xt4
                for t in range(T):
                    xt4 = xt4s[t // 4 * 4]
                    xt = xt4[:, (t % 4) * 128 : (t % 4 + 1) * 128]
                    z = ps.tile([H, 128], F32, tag="z")
                    nc.tensor.matmul(z[:], fc0w_sb[:], xt, start=True, stop=True)
                    h0t = sb.tile([H, 128], F32, tag="h0t")
                    nc.scalar.activation(
                        h0t[:], z[:], mybir.ActivationFunctionType.Relu, bias=b0_sb[:, 0:1]
                    )
                    nc.vector.tensor_copy(hT_bf[:, t * 128 : (t + 1) * 128], h0t[:])
                    ztr = ps.tile([128, H], F32, tag="ztr")
                    nc.tensor.transpose(ztr[:], h0t[:], ident[:])
                    nc.vector.tensor_copy(h_node[:, t * 128 : (t + 1) * 128], ztr[:])
                    hnb = sb.tile([128, H], BF16, tag="hnb")
                    nc.scalar.activation(
                        hnb[:], ztr[:], mybir.ActivationFunctionType.Copy
                    )
                    if t < TA:
                        nc.sync.dma_start(agin_a[0][t * 128 : (t + 1) * 128, :], hnb[:])
                    else:
                        o = (t - TA) * 128
                        nc.sync.dma_start(agin_b[0][o : o + 128, :], hnb[:])
                    if t == TA - 1:
                        ag(0, 0)
                if True:
                    ag(0, 1)

            # ---------------- conv layers ----------------
            for l in range(L):
                last = l == L - 1
                with (
                    tc.tile_pool(name=f"gsb{l}", bufs=3) as gp,
                    tc.tile_pool(name=f"ssb{l}", bufs=3) as sp,
                    tc.tile_pool(name=f"wsb{l}", bufs=4) as sb,
                    tc.tile_pool(name=f"ps{l}", bufs=1, space="PSUM") as ps1,
                    tc.tile_pool(name=f"ps2{l}", bufs=2, space="PSUM") as ps2,
                    tc.tile_pool(name=f"ps3{l}", bufs=2, space="PSUM") as ps3,
                ):
                    gtiles = [[None] * ncall[0], [None] * ncall[1]]
                    stiles = [[None] * ncall[0], [None] * ncall[1]]
                    for s, g in call_order:
                        st = sp.tile([128, BLK, 128], BF16, tag=f"S{s}")
                        nc.sync.dma_start(
                            st[:], s_io[s][:, g * BLK * 128 : (g + 1) * BLK * 128]
                        )
                        stiles[s][g] = st
                        # real index count for this call: the schedule only
                        # references chunks < tot[s], so the last call's
                        # rounding pad is never consumed and can be skipped
                        nreal = min(CH, int(tot[s]) * 128 - g * CH)
                        nblk = -(-nreal // 128)
                        gt = gp.tile([128, BLK, H], BF16, tag=f"G{s}")
                        src = (tab_lo[l] if s == 0 else tab_hi[l])[:, :]
                        nc.gpsimd.dma_gather(
                            gt[:, 0:nblk, :],
                            src,
                            idx_sb[s][:, g * (CH // 16) : (g + 1) * (CH // 16)],
                            num_idxs=nreal,
                            num_idxs_reg=nreal,
                            elem_size=H,
                            single_packet=(CH <= 1024),
                        )
                        gtiles[s][g] = gt

                    for t in range(T):
                        chunks = []
                        for s in range(2):
                            for j in range(NCH[t, s]):
                                chunks.append((s, int(base[t, s]) + j))
                        pg = ps2.tile([H, 128], F32, tag="gcn")
                        for j, (s, ch) in enumerate(chunks):
                            gt = gtiles[s][ch // BLK]
                            st = stiles[s][ch // BLK]
                            nc.tensor.matmul(
                                pg[:],
                                gt[:, ch % BLK, :],
                                st[:, ch % BLK, :],
                                start=(j == 0),
                                stop=(j == len(chunks) - 1),
                            )
                        nc.vector.tensor_copy(
                            gcn_all[:, t * 128 : (t + 1) * 128], pg[:]
                        )

                    for t in range(T):
                        hsl = hT_bf[:, t * 128 : (t + 1) * 128]
                        po = ps2.tile([128, K * H], F32, tag="O")
                        nc.tensor.matmul(
                            po[:],
                            gcn_all[:, t * 128 : (t + 1) * 128],
                            wtop_sb[l][:],
                            start=True,
                            stop=False,
                        )
                        nc.tensor.matmul(
                            po[:], hsl, wbot_sb[l][:], start=False, stop=True
                        )
                        pe = ps1.tile([128, K], F32, tag="e")
                        nc.tensor.matmul(pe[:], ones_sb[:], envb_sb[l][:], start=True, stop=False)
                        nc.tensor.matmul(pe[:], hsl, envw_sb[l][:], start=False, stop=True)
                        e_sb = sb.tile([128, K], F32, tag="e_sb")
                        nc.scalar.activation(
                            e_sb[:], pe[:], mybir.ActivationFunctionType.Exp
                        )
                        esum = sb.tile([128, 1], F32, tag="esum")
                        nc.vector.reduce_sum(esum[:], e_sb[:], axis=mybir.AxisListType.X)
                        nc.vector.reciprocal(esum[:], esum[:])

                        mixs = [sb.tile([128, H], F32, tag=f"mix{i}", name=f"mix{i}") for i in range(4)]
                        for k in range(K):
                            nc.scalar.activation(
                                mixs[k][:],
                                po[:, k * H : (k + 1) * H],
                                mybir.ActivationFunctionType.Copy,
                                scale=e_sb[:, k : k + 1],
                            )
                        nc.vector.tensor_add(mixs[0][:], mixs[0][:], mixs[1][:])
                        nc.vector.tensor_add(mixs[2][:], mixs[2][:], mixs[3][:])
                        nc.vector.tensor_add(mixs[0][:], mixs[0][:], mixs[2][:])
                        nc.scalar.activation(
                            mixs[1][:], mixs[0][:],
                            mybir.ActivationFunctionType.Copy,
                            scale=esum[:, 0:1],
                        )
                        hn = h_node[:, t * 128 : (t + 1) * 128]
                        nc.vector.tensor_add(mixs[1][:], mixs[1][:], hn)
                        nc.scalar.activation(
                            hn, mixs[1][:], mybir.ActivationFunctionType.Relu
                        )
                        ptr = ps3.tile([128, H], F32, tag="tr")
                        nc.tensor.transpose(ptr[:], hn, ident[:])
                        if not last:
                            nc.vector.tensor_copy(
                                hT_bf[:, t * 128 : (t + 1) * 128], ptr[:]
                            )
                            hnb = sb.tile([128, H], BF16, tag="hnb")
                            nc.scalar.activation(
                                hnb[:], hn, mybir.ActivationFunctionType.Copy
                            )
                            if t < TA:
                                nc.sync.dma_start(
                                    agin_a[1][t * 128 : (t + 1) * 128, :], hnb[:]
                                )
                            else:
                                o = (t - TA) * 128
                                nc.sync.dma_start(agin_b[1][o : o + 128, :], hnb[:])
                            if t == TA - 1:
                                ag(1, 0)
                            if t == T - 1:
                                ag(1, 1)
                        else:
                            h2T = sb.tile([H, 128], F32, tag="h2T")
                            nc.vector.tensor_copy(h2T[:], ptr[:])
                            pc = ps1.tile([128, C], F32, tag="c")
                            nc.tensor.matmul(
                                pc[:], h2T[:], fc1w_sb[:], start=True, stop=True
                            )
                            ob = sb.tile([128, C], F32, tag="ob")
                            nc.vector.tensor_add(ob[:], pc[:], b1_sb[:])
                            nc.sync.dma_start(
                                out_io[t * 128 : (t + 1) * 128, :], ob[:]
                            )
    nc.compile()
    return nc


def _in_maps(prep):
    maps = []
    for c in range(M):
        m = {
            "xT": prep["xT"][c],
            "fc0_w": prep["fc0_w"],
            "b0col": prep["b0"][:, None].copy(),
            "wtop": prep["wtop"],
            "wbot": prep["wbot"],
            "env_w": prep["env_w_bf"],
            "envb": prep["envb"].astype(np.float32),
            "fc1_w": prep["fc1_w"],
            "b1": prep["b1_bcast"],
        }
        for s in range(2):
            m[f"idx{s}"] = prep["idx_arr"][c, s]
            m[f"smat{s}"] = prep["s_arr"][c, s]
        maps.append(m)
    return maps


_compiled = {}


def _get_compiled(prep, key):
    if key not in _compiled:
        _compiled[key] = _build_program(prep)
    return _compiled[key]


def kernel(trace=False, **inputs):
    inputs = {k: np.asarray(v) for k, v in inputs.items()}
    prep = _preprocess(**inputs)
    key = hash(inputs["edge_index"].tobytes()) ^ hash(inputs["x"].tobytes()[:4096])
    nc = _get_compiled(prep, key)
    res = bass_utils.run_bass_kernel_spmd(
        nc, _in_maps(prep), core_ids=list(range(M)), trace=trace
    )
    out = np.zeros((N, C), np.float32)
    for c in range(M):
        out[c * NPC : (c + 1) * NPC] = res.results[c]["out"][:NPC]
    kernel.last_exec_time_ns = res.exec_time_ns
    kernel.last_results = res
    return out


# revision 15
# speedup vs baseline: 1.4952x; 1.0402x over previous
"""CaNet (moe_routing GNN) forward on 8 Trainium2 NeuronCores.

Sharding: nodes are range-partitioned across the 8 cores (6250 each, padded
to 6272 = 49*128). Each core owns the edges whose *destination* lands in its
node range. The GCN aggregation out[col] += val * h[row] is computed as a
sequence of tiny one-hot matmuls on the TensorEngine:

  - edges are sorted by destination tile (groups of 128 dest nodes) on the
    host and padded to multiples of 128 ("chunks");
  - the source features h[row] for one chunk are fetched from a replicated
    node-major bf16 table in HBM with the GpSimd dma_gather custom op
    (int16 indices; the table is split into a 24576-row "lo" region, the
    first 24 tiles of every core, and a 25600-row "hi" region, the last 25
    tiles of every core -- every chunk draws from one region);
  - the [128e x 128d] selection matrices S (S[e,d] = (d == ldest[e]) * val[e])
    are precomputed on the host (the edge list is a compile-time constant)
    and streamed from HBM with plain HWDGE DMA, one slab per gather call --
    this keeps the DVE out of the aggregation entirely;
  - psum_gcnT[f,d] += G_chunk.T @ S accumulates over the tile's chunks.

The lo/hi table split doubles as a latency hider: the AllGather between
layers is split into two collectives (tiles 0-23 -> tab_lo, tiles 24-48 ->
tab_hi), so the lo-region collective completes while the previous layer's
gathers still run and the next layer's lo gathers start immediately.

Dense per-node work (expert gate softmax, the K=4 expert convs, mixing,
residual relu, fc0/fc1) runs in bf16 matmuls + f32 psum per 128-node tile.

The per-core *program* is identical (SPMD); all per-core variation (gather
indices, S slabs, x slab) arrives via ExternalInputs. Chunk counts per
(tile, half) are max'd across cores so the schedule is static; padding
slots use idx=0 with an all-zero S row and contribute nothing.
"""

import sys

sys.path.insert(0, "/opt/trn_rl_repo")

import numpy as np
import ml_dtypes

import concourse.bacc as bacc
import concourse.tile as tile
import concourse.mybir as mybir
import concourse.bass as bass
from concourse import bass_utils
from concourse.masks import make_identity

# Problem constants (hardcoded per contract).
N = 50000
E = 800000
D = 128  # input dim
H = 128  # hidden dim
C = 47  # classes
K = 4  # experts
L = 2  # conv layers
M = 8  # cores

NPC = N // M  # 6250 nodes per core
T = (NPC + 127) // 128  # 49 tiles per core
NPAD = T * 128  # 6272
TA = 24  # tiles in the "lo" table region per core
TB = T - TA  # 25 tiles in the "hi" region
RA = TA * 128  # 3072 lo rows per core
RB = TB * 128  # 3200 hi rows per core
LOROW = M * RA  # 24576 lo region rows
HIROW = M * RB  # 25600 hi region rows
CH = 4096  # gather indices per dma_gather call
BLK = CH // 128  # 32 chunk blocks per gather call

F32 = mybir.dt.float32
BF16 = mybir.dt.bfloat16
I16 = mybir.dt.int16
BF = ml_dtypes.bfloat16


def _preprocess(x, edge_index, fc0_w, fc0_b, fc1_w, fc1_b, env_w, env_b, conv_w):
    """Host-side: degree/value computation, edge sort, static chunk schedule,
    per-core gather index + S-matrix arrays, weight packing."""
    row = np.asarray(edge_index[0], np.int64)
    col = np.asarray(edge_index[1], np.int64)

    deg = np.bincount(col, minlength=N).astype(np.float32)
    dinv = np.where(deg > 0, 1.0 / np.sqrt(deg), 0.0).astype(np.float32)
    val = (dinv[col] * dinv[row]).astype(np.float32)

    core = col // NPC
    dloc = col % NPC
    tl = dloc // 128
    ld = dloc % 128
    # table row: lo region holds tiles 0..TA-1 of every core, hi the rest
    score = row // NPC
    sloc = row % NPC
    half = (sloc >= RA).astype(np.int64)
    srow = np.where(half == 0, score * RA + sloc, score * RB + (sloc - RA))
    idx16 = srow  # already region-local

    cnt = np.bincount((core * T + tl) * 2 + half, minlength=M * T * 2).reshape(
        M, T, 2
    )
    mc = cnt.max(axis=0)  # [T, 2] unrounded static slot counts per tile
    assert mc.sum() > 0
    B = np.zeros((T, 2), np.int64)
    B[1:] = mc[:-1].cumsum(axis=0)  # slot base per tile, no 128-rounding
    totslot = mc.sum(axis=0)  # [2]
    ncall = [int(-(-int(totslot[s]) // CH)) for s in range(2)]
    totpad = [ncall[s] * BLK for s in range(2)]  # chunks incl. call padding

    # pieces: (tile, chunk) pairs in chunk-major order per stream, with the
    # within-call slab position of each
    pieces = [[], []]  # per stream: list of (t, c, pos)
    tile_pieces = [[] for _ in range(T)]  # per tile: (s, piece_index)
    for s in range(2):
        percall = {}
        plist = []
        for t in range(T):
            lo_c = int(B[t, s]) // 128
            hi_c = -(-(int(B[t, s]) + int(mc[t, s])) // 128)
            for c in range(lo_c, hi_c):
                pos = percall.get(c // BLK, 0)
                percall[c // BLK] = pos + 1
                tile_pieces[t].append((s, len(plist)))
                plist.append((t, c, pos))
        pieces[s] = plist
    PMAX = max(
        max((max(
            (pos + 1 for (t, c, pos) in pieces[s] if c // BLK == g), default=0
        ) for g in range(ncall[s])), default=0)
        for s in range(2)
    )
    PMAX = max(PMAX, 1)

    gkey = (core * 2 + half) * T + tl
    order = np.argsort(gkey, kind="stable")
    gsort = gkey[order]
    starts = np.searchsorted(gsort, np.arange(M * 2 * T))
    rank = np.arange(E, dtype=np.int64) - starts[gsort]
    slot = np.empty(E, np.int64)
    slot[order] = B[tl[order], half[order]] + rank

    idx_arr = np.zeros((M, 2), object)
    s_arr = np.zeros((M, 2), object)
    tlv = tl
    for c in range(M):
        for s in range(2):
            npad_s = totpad[s] * 128
            ia = np.zeros(npad_s, np.int16)
            sel = (core == c) & (half == s)
            ia[slot[sel]] = idx16[sel].astype(np.int16)
            # host-built per-piece selection matrices, slab layout
            # [128 e, ncall*PMAX, 128 d]
            sm = np.zeros((ncall[s] * PMAX, 128, 128), BF)
            sl = slot[sel]
            ldv = ld[sel]
            vv = val[sel].astype(BF)
            tv = tlv[sel]
            chv = sl // 128
            # map (tile, chunk) -> slab index
            p2slab = {}
            for (t, ch_, pos) in pieces[s]:
                p2slab[(t, ch_)] = (ch_ // BLK) * PMAX + pos
            slab_idx = np.array([p2slab[(t_, c_)] for t_, c_ in zip(tv, chv)])
            sm[slab_idx, sl % 128, ldv] = vv
            s_arr[c, s] = np.ascontiguousarray(
                sm.transpose(1, 0, 2).reshape(128, ncall[s] * PMAX * 128)
            )
            # wrap indices for dma_gather: per call [16, 512] tiled x8 -> [128, 512]
            iw = ia.reshape(ncall[s], CH // 16, 16)
            iw = np.transpose(iw, (0, 2, 1))  # [ncall, 16, 512]
            iw = np.tile(iw, (1, 8, 1))  # [ncall, 128, 512]
            idx_arr[c, s] = np.concatenate(list(iw), axis=1)  # [128, ncall*512]

    x = np.asarray(x, np.float32)
    xT = np.zeros((M, D, NPAD), np.float32)
    for c in range(M):
        xT[c, :, :NPC] = x[c * NPC : (c + 1) * NPC].T

    conv_w = np.asarray(conv_w, np.float32)
    wtop = np.zeros((L, H, K * H), BF)
    wbot = np.zeros((L, H, K * H), BF)
    for l in range(L):
        for k in range(K):
            wtop[l, :, k * H : (k + 1) * H] = conv_w[l, k, :H].astype(BF)
            wbot[l, :, k * H : (k + 1) * H] = conv_w[l, k, H:].astype(BF)
    env_w = np.asarray(env_w, np.float32)
    env_b = np.asarray(env_b, np.float32)
    prep = dict(
        pieces=pieces,
        tile_pieces=tile_pieces,
        PMAX=PMAX,
        totslot=totslot,
        ncall=ncall,
        totpad=totpad,
        idx_arr=idx_arr,
        s_arr=s_arr,
        xT=xT,
        fc0_w=np.asarray(fc0_w, np.float32),
        b0=np.asarray(fc0_b, np.float32),
        wtop=wtop,
        wbot=wbot,
        env_w_bf=env_w.astype(BF),
        envb=env_b.reshape(L, 1, K).copy(),
        fc1_w=np.asarray(fc1_w, np.float32),
        b1_bcast=np.tile(np.asarray(fc1_b, np.float32), (128, 1)),
    )
    return prep


def _emulate(prep):
    """Numpy mirror of the device program (validates schedule/indexing)."""
    ncall, totpad = prep["ncall"], prep["totpad"]
    pieces, tile_pieces, PMAX = prep["pieces"], prep["tile_pieces"], prep["PMAX"]
    h_node = np.zeros((M, NPAD, H), np.float32)
    for c in range(M):
        z = prep["xT"][c].T @ prep["fc0_w"] + prep["b0"]
        h_node[c] = np.maximum(z, 0.0)

    def build_tables(h_node):
        lo = np.concatenate([h_node[c, :RA].astype(BF) for c in range(M)])
        hi = np.concatenate([h_node[c, RA:].astype(BF) for c in range(M)])
        return [lo, hi]

    tabs = build_tables(h_node)

    for l in range(L):
        new_h = np.zeros_like(h_node)
        for c in range(M):
            G = [None, None]
            for s in range(2):
                ia = prep["idx_arr"][c, s]
                idxs = []
                for g in range(ncall[s]):
                    blkw = ia[:16, g * 512 : (g + 1) * 512]
                    idxs.append(blkw.T.reshape(-1))
                idxs = np.concatenate(idxs).astype(np.int64)
                G[s] = tabs[s][idxs].astype(np.float32)
            gcnT = np.zeros((T, H, 128), np.float32)
            for t in range(T):
                acc = np.zeros((H, 128), np.float32)
                for (s, pi) in tile_pieces[t]:
                    (_, ch, pos) = pieces[s][pi]
                    slab = (ch // BLK) * PMAX + pos
                    g = G[s][ch * 128 : (ch + 1) * 128]
                    S = (
                        prep["s_arr"][c, s][:, slab * 128 : (slab + 1) * 128]
                        .astype(np.float32)
                    )
                    acc += g.astype(BF).astype(np.float32).T @ S
                gcnT[t] = acc
            hT_bf = h_node[c].T.astype(BF)
            for t in range(T):
                sl = slice(t * 128, (t + 1) * 128)
                ht = hT_bf[:, sl].astype(np.float32)
                z = ht.T @ prep["env_w_bf"][l].astype(np.float32)
                e = np.exp(z + prep["envb"][l][0][None, :])
                e = e / e.sum(axis=1, keepdims=True)
                gt = gcnT[t].astype(BF).astype(np.float32)
                O = gt.T @ prep["wtop"][l].astype(np.float32) + ht.T @ prep[
                    "wbot"
                ][l].astype(np.float32)
                O = O.reshape(128, K, H)
                mixed = np.einsum("nk,nkh->nh", e, O)
                new_h[c, sl] = np.maximum(mixed + h_node[c, sl], 0.0)
        h_node = new_h
        tabs = build_tables(h_node)

    out = np.zeros((N, C), np.float32)
    for c in range(M):
        z = h_node[c] @ prep["fc1_w"] + prep["b1_bcast"][0][None, :]
        out[c * NPC : (c + 1) * NPC] = z[:NPC]
    return out


def _build_program(prep):
    ncall, totpad = prep["ncall"], prep["totpad"]
    pieces, tile_pieces, PMAX = prep["pieces"], prep["tile_pieces"], prep["PMAX"]
    totslot = prep["totslot"]
    nc = bacc.Bacc(
        "TRN2", target_bir_lowering=False, debug=False, num_devices=M
    )
    # I/O
    xT = nc.dram_tensor("xT", [D, NPAD], F32, kind="ExternalInput")
    idx_io = [
        nc.dram_tensor(f"idx{s}", [128, ncall[s] * (CH // 16)], I16, kind="ExternalInput")
        for s in range(2)
    ]
    s_io = [
        nc.dram_tensor(
            f"smat{s}", [128, ncall[s] * PMAX * 128], BF16, kind="ExternalInput"
        )
        for s in range(2)
    ]
    fc0_w = nc.dram_tensor("fc0_w", [D, H], F32, kind="ExternalInput")
    b0col = nc.dram_tensor("b0col", [H, 1], F32, kind="ExternalInput")
    wtop = nc.dram_tensor("wtop", [L, H, K * H], BF16, kind="ExternalInput")
    wbot = nc.dram_tensor("wbot", [L, H, K * H], BF16, kind="ExternalInput")
    env_w = nc.dram_tensor("env_w", [L, H, K], BF16, kind="ExternalInput")
    envb = nc.dram_tensor("envb", [L, 1, K], F32, kind="ExternalInput")
    fc1_w = nc.dram_tensor("fc1_w", [H, C], F32, kind="ExternalInput")
    b1 = nc.dram_tensor("b1", [128, C], F32, kind="ExternalInput")
    out_io = nc.dram_tensor("out", [NPAD, C], F32, kind="ExternalOutput")

    # internal DRAM: per-layer lo/hi gather tables + AllGather inputs
    tab_lo = [
        nc.dram_tensor(f"tlo{l}", [LOROW, H], BF16, kind="Internal", addr_space="Shared")
        for l in range(L)
    ]
    tab_hi = [
        nc.dram_tensor(f"thi{l}", [HIROW, H], BF16, kind="Internal", addr_space="Shared")
        for l in range(L)
    ]
    agin_a = [
        nc.dram_tensor(f"aga{l}", [RA, H], BF16, kind="Internal") for l in range(L)
    ]
    agin_b = [
        nc.dram_tensor(f"agb{l}", [RB, H], BF16, kind="Internal") for l in range(L)
    ]

    RG = [list(range(M))]

    def ag(l, which):
        if which == 0:
            nc.gpsimd.collective_compute(
                "AllGather", mybir.AluOpType.bypass, replica_groups=RG,
                ins=[agin_a[l][:]], outs=[tab_lo[l][:]],
            )
        else:
            nc.gpsimd.collective_compute(
                "AllGather", mybir.AluOpType.bypass, replica_groups=RG,
                ins=[agin_b[l][:]], outs=[tab_hi[l][:]],
            )

    # gather call issue order: lo leads by 2 calls so hi-region collectives
    # have time to land while lo gathers run
    call_order = []
    ig = [0, 0]
    while ig[0] < ncall[0] or ig[1] < ncall[1]:
        for s in range(2):
            want = ig[0] <= ig[1] + 2 if s == 0 else ig[1] < ig[0] - 1
            if ig[s] < ncall[s] and (want or ig[1 - s] >= ncall[1 - s]):
                call_order.append((s, ig[s]))
                ig[s] += 1

    with tile.TileContext(nc) as tc:
        with tc.tile_pool(name="const", bufs=1) as const:
            ident = const.tile([128, 128], F32)
            make_identity(nc, ident[:])
            fc0w_sb = const.tile([D, H], F32)
            nc.sync.dma_start(fc0w_sb[:], fc0_w[:])
            b0_sb = const.tile([H, 1], F32)
            nc.sync.dma_start(b0_sb[:], b0col[:])
            wtop_sb = [const.tile([H, K * H], BF16, tag=f"wtop{l}", name=f"wtop{l}") for l in range(L)]
            wbot_sb = [const.tile([H, K * H], BF16, tag=f"wbot{l}", name=f"wbot{l}") for l in range(L)]
            envw_sb = [const.tile([H, K], BF16, tag=f"envw{l}", name=f"envw{l}") for l in range(L)]
            envb_sb = [const.tile([1, K], F32, tag=f"envb{l}", name=f"envb{l}") for l in range(L)]
            ones_sb = const.tile([1, 128], F32)
            nc.vector.memset(ones_sb[:], 1.0)
            for l in range(L):
                nc.sync.dma_start(wtop_sb[l][:], wtop[l])
                nc.sync.dma_start(wbot_sb[l][:], wbot[l])
                nc.sync.dma_start(envw_sb[l][:], env_w[l])
                nc.sync.dma_start(envb_sb[l][:], envb[l])
            fc1w_sb = const.tile([H, C], F32)
            nc.sync.dma_start(fc1w_sb[:], fc1_w[:])
            b1_sb = const.tile([128, C], F32)
            nc.sync.dma_start(b1_sb[:], b1[:])
            idx_sb = [
                const.tile([128, ncall[s] * (CH // 16)], I16, tag=f"idx{s}", name=f"idxsb{s}")
                for s in range(2)
            ]
            for s in range(2):
                nc.sync.dma_start(idx_sb[s][:], idx_io[s][:])
            hT_bf = const.tile([H, NPAD], BF16)  # feat-major h (matmul operand)
            h_node = const.tile([128, T * 128], F32)  # node-major h blocks
            gcn_all = const.tile([H, T * 128], BF16)  # aggregated gcn per tile

            # ---------------- fc0 ----------------
            with (
                tc.tile_pool(name="fc0sb", bufs=3) as sb,
                tc.tile_pool(name="fc0ps", bufs=3, space="PSUM") as ps,
            ):
                xt4s = {}
                for tb in range(0, T, 4):
                    nt = min(4, T - tb)
                    xt4 = sb.tile([D, 512], F32, tag="xt")
                    nc.sync.dma_start(
                        xt4[:, 0 : nt * 128],
                        xT[:, tb * 128 : (tb + nt) * 128],
                    )
                    xt4s[tb] = xt4
                for t in range(T):
                    xt4 = xt4s[t // 4 * 4]
                    xt = xt4[:, (t % 4) * 128 : (t % 4 + 1) * 128]
                    z = ps.tile([H, 128], F32, tag="z")
                    nc.tensor.matmul(z[:], fc0w_sb[:], xt, start=True, stop=True)
                    h0t = sb.tile([H, 128], F32, tag="h0t")
                    nc.scalar.activation(
                        h0t[:], z[:], mybir.ActivationFunctionType.Relu, bias=b0_sb[:, 0:1]
                    )
                    nc.vector.tensor_copy(hT_bf[:, t * 128 : (t + 1) * 128], h0t[:])
                    ztr = ps.tile([128, H], F32, tag="ztr")
                    nc.tensor.transpose(ztr[:], h0t[:], ident[:])
                    nc.vector.tensor_copy(h_node[:, t * 128 : (t + 1) * 128], ztr[:])
                    hnb = sb.tile([128, H], BF16, tag="hnb")
                    nc.scalar.activation(
                        hnb[:], ztr[:], mybir.ActivationFunctionType.Copy
                    )
                    if t < TA:
                        nc.sync.dma_start(agin_a[0][t * 128 : (t + 1) * 128, :], hnb[:])
                    else:
                        o = (t - TA) * 128
                        nc.sync.dma_start(agin_b[0][o : o + 128, :], hnb[:])
                    if t == TA - 1:
                        ag(0, 0)
                if True:
                    ag(0, 1)

            # ---------------- conv layers ----------------
            for l in range(L):
                last = l == L - 1
                with (
                    tc.tile_pool(name=f"gsb{l}", bufs=3) as gp,
                    tc.tile_pool(name=f"ssb{l}", bufs=3) as sp,
                    tc.tile_pool(name=f"wsb{l}", bufs=4) as sb,
                    tc.tile_pool(name=f"ps{l}", bufs=1, space="PSUM") as ps1,
                    tc.tile_pool(name=f"ps2{l}", bufs=2, space="PSUM") as ps2,
                    tc.tile_pool(name=f"ps3{l}", bufs=2, space="PSUM") as ps3,
                ):
                    gtiles = [[None] * ncall[0], [None] * ncall[1]]
                    stiles = [[None] * ncall[0], [None] * ncall[1]]
                    for s, g in call_order:
                        st = sp.tile([128, PMAX, 128], BF16, tag=f"S{s}")
                        nc.sync.dma_start(
                            st[:],
                            s_io[s][:, g * PMAX * 128 : (g + 1) * PMAX * 128],
                        )
                        stiles[s][g] = st
                        nreal = min(CH, int(totslot[s]) - g * CH)
                        nblk = -(-nreal // 128)
                        gt = gp.tile([128, BLK, H], BF16, tag=f"G{s}")
                        src = (tab_lo[l] if s == 0 else tab_hi[l])[:, :]
                        nc.gpsimd.dma_gather(
                            gt[:, 0:nblk, :],
                            src,
                            idx_sb[s][:, g * (CH // 16) : (g + 1) * (CH // 16)],
                            num_idxs=nreal,
                            num_idxs_reg=nreal,
                            elem_size=H,
                            single_packet=(CH <= 1024),
                        )
                        gtiles[s][g] = gt

                    for t in range(T):
                        plist = tile_pieces[t]
                        pg = ps2.tile([H, 128], F32, tag="gcn")
                        for j, (s, pi) in enumerate(plist):
                            (_, ch, pos) = pieces[s][pi]
                            gt = gtiles[s][ch // BLK]
                            st = stiles[s][ch // BLK]
                            nc.tensor.matmul(
                                pg[:],
                                gt[:, ch % BLK, :],
                                st[:, pos, :],
                                start=(j == 0),
                                stop=(j == len(plist) - 1),
                            )
                        nc.vector.tensor_copy(
                            gcn_all[:, t * 128 : (t + 1) * 128], pg[:]
                        )

                    for t in range(T):
                        hsl = hT_bf[:, t * 128 : (t + 1) * 128]
                        po = ps2.tile([128, K * H], F32, tag="O")
                        nc.tensor.matmul(
                            po[:],
                            gcn_all[:, t * 128 : (t + 1) * 128],
                            wtop_sb[l][:],
                            start=True,
                            stop=False,
                        )
                        nc.tensor.matmul(
                            po[:], hsl, wbot_sb[l][:], start=False, stop=True
                        )
                        pe = ps1.tile([128, K], F32, tag="e")
                        nc.tensor.matmul(pe[:], ones_sb[:], envb_sb[l][:], start=True, stop=False)
                        nc.tensor.matmul(pe[:], hsl, envw_sb[l][:], start=False, stop=True)
                        e_sb = sb.tile([128, K], F32, tag="e_sb")
                        nc.scalar.activation(
                            e_sb[:], pe[:], mybir.ActivationFunctionType.Exp
                        )
                        esum = sb.tile([128, 1], F32, tag="esum")
                        nc.vector.reduce_sum(esum[:], e_sb[:], axis=mybir.AxisListType.X)
                        nc.vector.reciprocal(esum[:], esum[:])

                        mixs = [sb.tile([128, H], F32, tag=f"mix{i}", name=f"mix{i}") for i in range(4)]
                        for k in range(K):
                            nc.scalar.activation(
                                mixs[k][:],
                                po[:, k * H : (k + 1) * H],
                                mybir.ActivationFunctionType.Copy,
                                scale=e_sb[:, k : k + 1],
                            )
                        nc.vector.tensor_add(mixs[0][:], mixs[0][:], mixs[1][:])
                        nc.vector.tensor_add(mixs[2][:], mixs[2][:], mixs[3][:])
                        nc.vector.tensor_add(mixs[0][:], mixs[0][:], mixs[2][:])
                        nc.scalar.activation(
                            mixs[1][:], mixs[0][:],
                            mybir.ActivationFunctionType.Copy,
                            scale=esum[:, 0:1],
                        )
                        hn = h_node[:, t * 128 : (t + 1) * 128]
                        nc.vector.tensor_add(mixs[1][:], mixs[1][:], hn)
                        nc.scalar.activation(
                            hn, mixs[1][:], mybir.ActivationFunctionType.Relu
                        )
                        ptr = ps3.tile([128, H], F32, tag="tr")
                        nc.tensor.transpose(ptr[:], hn, ident[:])
                        if not last:
                            nc.vector.tensor_copy(
                                hT_bf[:, t * 128 : (t + 1) * 128], ptr[:]
                            )
                            hnb = sb.tile([128, H], BF16, tag="hnb")
                            nc.scalar.activation(
                                hnb[:], hn, mybir.ActivationFunctionType.Copy
                            )
                            if t < TA:
                                nc.sync.dma_start(
                                    agin_a[1][t * 128 : (t + 1) * 128, :], hnb[:]
                                )
                            else:
                                o = (t - TA) * 128
                                nc.sync.dma_start(agin_b[1][o : o + 128, :], hnb[:])
                            if t == TA - 1:
                                ag(1, 0)
                            if t == T - 1:
                                ag(1, 1)
                        else:
                            h2T = sb.tile([H, 128], F32, tag="h2T")
                            nc.vector.tensor_copy(h2T[:], ptr[:])
                            pc = ps1.tile([128, C], F32, tag="c")
                            nc.tensor.matmul(
                                pc[:], h2T[:], fc1w_sb[:], start=True, stop=True
                            )
                            ob = sb.tile([128, C], F32, tag="ob")
                            nc.vector.tensor_add(ob[:], pc[:], b1_sb[:])
                            nc.sync.dma_start(
                                out_io[t * 128 : (t + 1) * 128, :], ob[:]
                            )
    nc.compile()
    return nc


def _in_maps(prep):
    maps = []
    for c in range(M):
        m = {
            "xT": prep["xT"][c],
            "fc0_w": prep["fc0_w"],
            "b0col": prep["b0"][:, None].copy(),
            "wtop": prep["wtop"],
            "wbot": prep["wbot"],
            "env_w": prep["env_w_bf"],
            "envb": prep["envb"].astype(np.float32),
            "fc1_w": prep["fc1_w"],
            "b1": prep["b1_bcast"],
        }
        for s in range(2):
            m[f"idx{s}"] = prep["idx_arr"][c, s]
            m[f"smat{s}"] = prep["s_arr"][c, s]
        maps.append(m)
    return maps


_compiled = {}


def _get_compiled(prep, key):
    if key not in _compiled:
        _compiled[key] = _build_program(prep)
    return _compiled[key]


def kernel(trace=False, **inputs):
    inputs = {k: np.asarray(v) for k, v in inputs.items()}
    prep = _preprocess(**inputs)
    key = hash(inputs["edge_index"].tobytes()) ^ hash(inputs["x"].tobytes()[:4096])
    nc = _get_compiled(prep, key)
    res = bass_utils.run_bass_kernel_spmd(
        nc, _in_maps(prep), core_ids=list(range(M)), trace=trace
    )
    out = np.zeros((N, C), np.float32)
    for c in range(M):
        out[c * NPC : (c + 1) * NPC] = res.results[c]["out"][:NPC]
    kernel.last_exec_time_ns = res.exec_time_ns
    kernel.last_results = res
    return out
